# revision 3
# baseline (speedup 1.0000x reference)
"""2-layer GAT (nn_GAT_31490700214331) on 8 Trainium2 NeuronCores.

Strategy (dst-sharded, SPMD, per-core-rotated node layout):
  - Nodes are block-partitioned: core c owns nodes [c*6250, (c+1)*6250).
  - Every table on core c uses a ROTATED row order: node n lives at row
    (n - c*6250) mod 50000, so each core's own nodes are rows 0..6249 and
    the single SPMD program has no core-dependent offsets — the rotation
    lives entirely in host-prepared index/input arrays.
  - Layer-0 features (h0 = x @ W0) + attention alphas are computed
    replicated on every core (cheap) into a rotated DRAM table; edges are
    grouped by dst tile (128 dsts) and their source rows fetched with
    dma_gather (int16 indices -> the table is gathered through two views,
    rows [0, SPLIT) and [SPLIT, ...), keeping indices < 32768).
  - Per-edge dst alphas come from ONE merged dma_gather over a 256B column
    window of the lo table (local dst rows < 6250 < 32768, so no lo/hi
    split and no separate alpha tables).
  - Edge softmax (safe without segment-max: |e| <= ~5) and the weighted
    aggregation are fused into per-chunk 128x128 incidence matmuls
    accumulating in PSUM; denominators ride along as 8 extra columns.
  - Between layers the ELU'd hidden state is AllGather'd (feature-major),
    rotated into per-core order with partition-id-offset DMA copies, and
    layer 1 repeats the scheme with 512-wide features and a head-mean +
    log_softmax epilogue.
  - alpha projections fold into the weight matmuls on the host:
    h @ blockdiag(a) == x @ (W @ blockdiag(a)), so the device gets
    W0a=[256,16] / W1a=[128,16] and computes alphas as 16 extra psum cols.

Self-contained: call kernel(**inputs) with the full-problem arrays.
"""
import numpy as np
from contextlib import ExitStack

import concourse.bacc as bacc
import concourse.bass as bass
import concourse.mybir as mybir
from concourse.tile import TileContext
from concourse.bass_utils import run_bass_kernel_spmd

F16 = mybir.dt.float16
F32 = mybir.dt.float32
I16 = mybir.dt.int16

N = 50000
NFEAT = 256
NHID = 128
NCLASS = 64
HEADS = 8
SLOPE = 0.2
NCORES = 8
NLOC = N // NCORES           # 6250
LT = (NLOC + 127) // 128     # 49 local dst tiles
LAST_ROWS = NLOC - (LT - 1) * 128   # 106 rows in the last tile
GT = 392                     # global node tiles (392*128 = 50176)
GROWS = GT * 128
SPLIT = 25000                # low/high gather-table split (4 core blocks)
CCOLS = 1536                 # collective chunk width (12 B-tiles)
NCHUNK = 5                   # 4 full chunks + 106-col tail
SENT = 300.0                 # dst_rel sentinel for padding slots
T0W = 256                    # t0 row: [h0(128)|as0(8)|ad0(8)|junk] f16
T1W = 640                    # t1 row: [h1(512)|as1(8)|ad1(8)|junk] f16

_cache = {}


# --------------------------------------------------------------------------
# host-side preparation
# --------------------------------------------------------------------------

def _wrap_idx(idx):
    """[n] int -> [128, n//16] int16 wrapped gather-index layout."""
    n = idx.shape[0]
    assert n % 16 == 0
    w = idx.reshape(n // 16, 16).T.astype(np.int16)
    return np.tile(w, (8, 1))


def _prep_edges(src, dst):
    cores = []
    for c in range(NCORES):
        m = (dst >= c * NLOC) & (dst < (c + 1) * NLOC)
        s = src[m].astype(np.int64)
        d = dst[m].astype(np.int64) - c * NLOC
        order = np.argsort(d, kind="stable")
        s, d = s[order], d[order]
        s_rot = (s - c * NLOC) % N
        tiles = []
        for t in range(LT):
            sel = (d >= t * 128) & (d < (t + 1) * 128)
            st, dt = s_rot[sel], d[sel] - t * 128
            lo = st < SPLIT
            tiles.append((st[lo], dt[lo], st[~lo] - SPLIT, dt[~lo]))
        cores.append(tiles)
    nl = max(len(t[0]) for tl in cores for t in tl)
    nh = max(len(t[2]) for tl in cores for t in tl)
    NL = max(1, (nl + 127) // 128)
    NH = max(1, (nh + 127) // 128)
    assert NL * 128 <= 1024 and NH * 128 <= 1024, (NL, NH)
    CH = NL + NH

    out = []
    for c in range(NCORES):
        eil = np.zeros((LT, 128, NL * 8), np.int16)
        eih = np.zeros((LT, 128, NH * 8), np.int16)
        ea = np.zeros((LT, 128, CH * 8), np.int16)
        drel = np.full((LT, 128, CH * 8), SENT, np.float16)
        for t in range(LT):
            sl, dl, sh, dh = cores[c][t]
            il = np.zeros(NL * 128, np.int64)
            il[: len(sl)] = sl
            ih = np.zeros(NH * 128, np.int64)
            ih[: len(sh)] = sh
            # merged dst-row indices for the alpha window gather: local dst
            # rows (< 6250) for lo-edge slots then hi-edge slots
            aa = np.zeros(CH * 128, np.int64)
            aa[: len(dl)] = t * 128 + dl
            aa[NL * 128: NL * 128 + len(dh)] = t * 128 + dh
            eil[t] = _wrap_idx(il)
            eih[t] = _wrap_idx(ih)
            ea[t] = _wrap_idx(aa)
            rl = np.full(NL * 128, SENT)
            rl[: len(dl)] = dl
            rh = np.full(NH * 128, SENT)
            rh[: len(dh)] = dh
            r = np.concatenate([rl, rh]).reshape(CH, 128).T
            # replicate x8 so the is_equal inc build has packed operands
            drel[t] = np.broadcast_to(
                r.astype(np.float16)[:, :, None],
                (128, CH, 8)).reshape(128, CH * 8)
        epack = np.concatenate([eil, eih, ea, drel.view(np.int16)], axis=2)
        out.append(dict(epack=np.ascontiguousarray(epack)))
    return NL, NH, out


def _prep_inputs(x, edge_index, W0, a_src0, a_dst0, b0, W1, a_src1, a_dst1,
                 b1):
    src = np.asarray(edge_index[0]).astype(np.int64)
    dst = np.asarray(edge_index[1]).astype(np.int64)
    NL, NH, edata = _prep_edges(src, dst)

    def bd(a):  # [H, D] -> blockdiag [H*D, H]
        a = np.asarray(a, np.float32)
        H, D = a.shape
        m = np.zeros((H * D, H), np.float32)
        for h in range(H):
            m[h * D:(h + 1) * D, h] = a[h]
        return m

    W0 = np.asarray(W0, np.float32)
    W1 = np.asarray(W1, np.float32)
    W0a = np.concatenate([W0 @ bd(a_src0), W0 @ bd(a_dst0)], 1)  # [256, 16]
    # head-innermost feature interleave: new col d*8+h <- old col h*D+d
    perm0 = np.array([(f % 8) * 16 + f // 8 for f in range(128)])
    perm1 = np.array([(f % 8) * 64 + f // 8 for f in range(512)])
    W0cat = np.concatenate([W0[:, perm0], W0a], 1)               # [256, 144]
    W1a = np.concatenate([W1 @ bd(a_src1), W1 @ bd(a_dst1)], 1)  # [128, 16]

    x = np.asarray(x, np.float32)
    ident = np.eye(128, dtype=np.float16)
    colio = np.tile(np.arange(128, dtype=np.float16)[None, :], (128, 1))
    b0b = np.tile(np.asarray(b0, np.float32)[None, :], (128, 1))
    b1b = np.tile(np.asarray(b1, np.float32)[None, :], (128, 1))

    in_maps = []
    for c in range(NCORES):
        rot = np.roll(np.arange(N), -c * NLOC)
        xr = np.zeros((GROWS, NFEAT), np.float16)
        xr[:N] = x[rot].astype(np.float16)
        # [gg, 128(j feat), 2(g), 2(k), 128(p node)]: partition = feature,
        # per-partition contiguous 1KB runs
        xtt = (xr.reshape(GROWS // 256, 2, 128, 2, 128)
               .transpose(0, 4, 1, 3, 2))
        m = dict(
            xT=np.ascontiguousarray(xtt),
            W0=np.ascontiguousarray(
                W0cat.astype(np.float16).reshape(2, 128, NHID + 16)),
            W1=np.ascontiguousarray(W1[perm0][:, perm1].astype(np.float16)),
            W1a=np.ascontiguousarray(W1a[perm0].astype(np.float16)),
            b0b=np.ascontiguousarray(b0b[:, perm0]), b1b=b1b,
            ident=ident, colio=colio,
            **edata[c],
        )
        in_maps.append(m)
    return NL, NH, in_maps


# --------------------------------------------------------------------------
# device program
# --------------------------------------------------------------------------

def build(NL, NH, lt=LT, gt=GT, debug=False, phases="ABCDE",
          sim_safe=False):
    CH = NL + NH
    HID16 = NHID + 16
    NLI = NL * 128
    NHI = NH * 128

    EPW = CH * 24   # packed int16 cols: il|ih, ea, drel8
    nc = bacc.Bacc("TRN2")
    xT = nc.dram_tensor("xT", [GROWS // 256, 128, 2, 2, 128], F16,
                        kind="ExternalInput")
    W0i = nc.dram_tensor("W0", [2, 128, NHID + 16], F16,
                         kind="ExternalInput")
    W1i = nc.dram_tensor("W1", [NHID, 512], F16, kind="ExternalInput")
    W1ai = nc.dram_tensor("W1a", [NHID, 16], F16, kind="ExternalInput")
    b0bi = nc.dram_tensor("b0b", [128, NHID], F32, kind="ExternalInput")
    b1bi = nc.dram_tensor("b1b", [128, NCLASS], F32, kind="ExternalInput")
    identi = nc.dram_tensor("ident", [128, 128], F16, kind="ExternalInput")
    colioi = nc.dram_tensor("colio", [128, 128], F16, kind="ExternalInput")
    epacki = nc.dram_tensor("epack", [lt, 128, EPW], I16,
                            kind="ExternalInput")
    out = nc.dram_tensor("out", [NLOC, NCLASS], F32, kind="ExternalOutput")

    with TileContext(nc) as tc, ExitStack() as stk:
        reg_l = nc.gpsimd.to_reg(NLI)
        reg_h = nc.gpsimd.to_reg(NHI)
        dpool = stk.enter_context(
            tc.tile_pool(name="dram", bufs=1, space="DRAM"))
        t0lo = dpool.tile([SPLIT, T0W], F16, tag="t0lo")
        t0hi = dpool.tile([GROWS - SPLIT, T0W], F16, tag="t0hi")
        t1lo = dpool.tile([SPLIT, T1W], F16, tag="t1lo")
        t1hi = dpool.tile([GROWS - SPLIT, T1W], F16, tag="t1hi")
        CW = [CCOLS] * 4 + [NLOC - 4 * CCOLS]   # 4x1536 + 106
        aginc = [dpool.tile([128, CW[k]], F16, tag=f"agin{k}",
                            name=f"agin{k}")
                 for k in range(NCHUNK)]
        agoutc = [dpool.tile([NCORES * 128, CW[k]], F16, tag=f"agout{k}",
                             addr_space="Shared", name=f"agout{k}")
                  for k in range(NCHUNK)]

        cpool = stk.enter_context(tc.tile_pool(name="const", bufs=1))
        W0s = cpool.tile([128, 2, NHID + 16], F16)
        nc.sync.dma_start(out=W0s[:], in_=W0i.rearrange("k p n -> p k n"))
        W1s = cpool.tile([128, 512], F16)
        nc.sync.dma_start(out=W1s[:], in_=W1i[:])
        W1as = cpool.tile([128, 16], F16)
        nc.sync.dma_start(out=W1as[:], in_=W1ai[:])
        b0s = cpool.tile([128, NHID], F32)
        nc.sync.dma_start(out=b0s[:], in_=b0bi[:])
        b1s = cpool.tile([128, NCLASS], F32)
        nc.sync.dma_start(out=b1s[:], in_=b1bi[:])
        idents = cpool.tile([128, 128], F16)
        nc.sync.dma_start(out=idents[:], in_=identi[:])
        colios = cpool.tile([128, 128], F16)
        nc.sync.dma_start(out=colios[:], in_=colioi[:])

        pid = nc.partition_id(engines=[mybir.EngineType.SP])
        sregs = [nc.sync.snap(((j + pid) % NCORES) * 128)
                 for j in range(NCORES)]

        # ---------------- phase A: layer-0 tables (replicated) ------------
        with ExitStack() as pa:
            xp = pa.enter_context(tc.tile_pool(name="pa_x", bufs=4))
            pp = pa.enter_context(
                tc.tile_pool(name="pa_ps", bufs=2, space="PSUM"))
            rp = pa.enter_context(tc.tile_pool(name="pa_row", bufs=4))
            assert gt % 2 == 0
            for gg in range(gt // 2):
                xa = xp.tile([128, 2, 2, 128], F16, tag="xa")
                nc.sync.dma_start(out=xa[:], in_=xT[gg])
                row = rp.tile([128, 2, T0W], F16, tag="row")
                for g2 in range(2):
                    ps = pp.tile([128, HID16], F32, tag=f"ps{g2}")
                    for k in range(2):
                        nc.tensor.matmul(ps[:], xa[:, g2, k, :],
                                         W0s[:, k, :],
                                         start=(k == 0), stop=(k == 1))
                    eng2 = nc.vector.tensor_copy if g2 else nc.scalar.copy
                    eng2(row[:, g2, 0:HID16], ps[:])
                eng = nc.scalar if gg % 2 else nc.sync
                g0 = 2 * gg * 128
                if g0 + 256 <= SPLIT:
                    eng.dma_start(
                        out=t0lo[g0:g0 + 256, 0:HID16]
                        .rearrange("(g p) w -> p g w", p=128),
                        in_=row[:, :, 0:HID16])
                elif g0 >= SPLIT:
                    o = g0 - SPLIT
                    eng.dma_start(
                        out=t0hi[o:o + 256, 0:HID16]
                        .rearrange("(g p) w -> p g w", p=128),
                        in_=row[:, :, 0:HID16])
                else:
                    # group straddles the lo/hi split inside tile g2=1
                    cut = SPLIT - g0 - 128   # rows of g2=1 going to lo
                    eng.dma_start(out=t0lo[g0:g0 + 128, 0:HID16],
                                  in_=row[:, 0, 0:HID16])
                    eng.dma_start(
                        out=t0lo[g0 + 128:SPLIT, 0:HID16],
                        in_=row[0:cut, 1, 0:HID16])
                    eng.dma_start(
                        out=t0hi[0:256 - 128 - cut, 0:HID16],
                        in_=row[cut:128, 1, 0:HID16])

        # ---------------- shared edge phase -------------------------------
        def edge_phase(layer, tbl_lo, tbl_hi, awin, aoff, fdim, trow, grow,
                       post_fn, fin, hook=None):
            o_il, o_ih = 0, NL * 8
            o_ea = CH * 8
            o_dr = CH * 16
            with ExitStack() as pb:
                ip = pb.enter_context(
                    tc.tile_pool(name=f"ix{layer}", bufs=4))
                gp = pb.enter_context(
                    tc.tile_pool(name=f"gg{layer}", bufs=4))
                apl = pb.enter_context(
                    tc.tile_pool(name=f"ga{layer}", bufs=3))
                rp2 = pb.enter_context(
                    tc.tile_pool(name=f"rh{layer}", bufs=3))
                pp2 = pb.enter_context(
                    tc.tile_pool(name=f"ps{layer}", bufs=2, space="PSUM"))
                op = pb.enter_context(
                    tc.tile_pool(name=f"po{layer}", bufs=3))
                for t in range(lt):
                    ep = ip.tile([128, EPW], I16, tag="ep")
                    nc.sync.dma_start(out=ep[:], in_=epacki[t])
                    il = ep[:, o_il:o_il + NL * 8]
                    ih = ep[:, o_ih:o_ih + NH * 8]
                    ea = ep[:, o_ea:o_ea + CH * 8]
                    dr8 = (ep[:, o_dr:o_dr + CH * 8].bitcast(F16)
                           .rearrange("p (c e) -> p c e", e=8))

                    G = gp.tile([128, CH, grow], F16, tag="G")
                    nc.gpsimd.dma_gather(G[:, 0:NL, :], tbl_lo[:], il,
                                         NLI, reg_l, grow, elem_step=trow)
                    nc.gpsimd.dma_gather(G[:, NL:CH, :], tbl_hi[:],
                                         ih, NHI, reg_h, grow,
                                         elem_step=trow)
                    # dst-alpha gathers: 256B column window of the lo table
                    # (local dst rows < 32768; <=1024 idx per call to fit
                    # the 1024-descriptor SWDGE ring)
                    A = apl.tile([128, CH, 128], F16, tag="A")
                    nc.gpsimd.dma_gather(A[:, 0:NL, :], awin,
                                         ea[:, 0:NL * 8], NLI, reg_l, 128,
                                         elem_step=trow)
                    nc.gpsimd.dma_gather(A[:, NL:CH, :], awin,
                                         ea[:, NL * 8:CH * 8], NHI, reg_h,
                                         128, elem_step=trow)

                    inc = rp2.tile([128, CH, 128], F16, tag="inc")
                    nc.vector.tensor_tensor(
                        out=inc[:].rearrange("p c (g e) -> p c g e", e=8),
                        in0=dr8.unsqueeze(2)
                        .broadcast_to([128, CH, 16, 8]),
                        in1=colios[:].rearrange("p (g e) -> p g e", e=8)
                        .unsqueeze(1).broadcast_to([128, CH, 16, 8]),
                        op=mybir.AluOpType.is_equal)
                    EX = rp2.tile([128, CH, 8], F16, tag="EX")
                    nc.vector.tensor_tensor(
                        out=EX[:], in0=G[:, :, fdim:fdim + 8],
                        in1=A[:, :, aoff:aoff + 8], op=mybir.AluOpType.add)
                    if sim_safe:
                        # interp executor lacks Prelu: 0.2*v + relu(0.8*v)
                        EXr = rp2.tile([128, CH, 8], F16, tag="EXr")
                        nc.scalar.activation(
                            EXr[:], EX[:],
                            mybir.ActivationFunctionType.Relu, scale=0.8)
                        nc.vector.tensor_scalar_mul(EX[:], EX[:], SLOPE)
                        nc.vector.tensor_tensor(
                            out=EX[:], in0=EX[:], in1=EXr[:],
                            op=mybir.AluOpType.add)
                    else:
                        nc.scalar.activation(
                            EX[:], EX[:],
                            mybir.ActivationFunctionType.Prelu, alpha=SLOPE)
                    nc.scalar.activation(
                        EX[:], EX[:], mybir.ActivationFunctionType.Exp)

                    R = rp2.tile([128, CH, fdim], F16, tag="R")
                    H = HEADS
                    D = fdim // H
                    nc.vector.tensor_tensor(
                        out=R[:, :, 0:fdim]
                        .rearrange("p c (d h) -> p c d h", h=H),
                        in0=G[:, :, 0:fdim]
                        .rearrange("p c (d h) -> p c d h", h=H),
                        in1=EX[:].unsqueeze(2).broadcast_to([128, CH, D, H]),
                        op=mybir.AluOpType.mult)

                    P1 = pp2.tile([128, fdim], F32, tag="P1")
                    P2 = pp2.tile([128, 8], F32, tag="P2")
                    for ch in range(CH):
                        nc.tensor.matmul(P1[:], inc[:, ch, :],
                                         R[:, ch, 0:fdim],
                                         start=(ch == 0),
                                         stop=(ch == CH - 1))
                    for ch in range(CH):
                        nc.tensor.matmul(P2[:], inc[:, ch, :],
                                         EX[:, ch, :],
                                         start=(ch == 0),
                                         stop=(ch == CH - 1))
                    post_fn(t, P1, P2, op, pp2, fin)
                    if hook is not None:
                        hook(t)

        # ---- L0 post: softmax-div, +b0, ELU, transpose, store ------------
        def post0(t, P1, P2, op, pp2, fin):
            rows = 128 if t < lt - 1 else LAST_ROWS
            r8 = op.tile([128, 8], F32, tag="r8")
            nc.vector.tensor_scalar_add(r8[:], P2[:], 1e-16)
            nc.vector.reciprocal(r8[:], r8[:])
            z = op.tile([128, NHID], F32, tag="z")
            nc.vector.tensor_tensor(
                out=z[:].rearrange("p (d h) -> p d h", h=HEADS),
                in0=P1[:].rearrange("p (d h) -> p d h", h=HEADS),
                in1=r8[:].unsqueeze(1).broadcast_to([128, 16, HEADS]),
                op=mybir.AluOpType.mult)
            nc.vector.tensor_tensor(out=z[:], in0=z[:], in1=b0s[:],
                                    op=mybir.AluOpType.add)
            zm = op.tile([128, NHID], F32, tag="zm")
            nc.vector.tensor_scalar_min(zm[:], z[:], 0.0)
            nc.scalar.activation(zm[:], zm[:],
                                 mybir.ActivationFunctionType.Exp)
            zp = op.tile([128, NHID], F32, tag="zp")
            nc.vector.tensor_scalar_max(zp[:], z[:], 0.0)
            nc.vector.tensor_tensor(out=zp[:], in0=zp[:], in1=zm[:],
                                    op=mybir.AluOpType.add)
            h1 = op.tile([128, NHID], F16, tag="h1")
            nc.vector.tensor_scalar_add(h1[:], zp[:], -1.0)
            pst = pp2.tile([128, 128], F16, tag="pst")
            nc.tensor.transpose(pst[:], h1[:], idents[:])
            hT = op.tile([128, 128], F16, tag="hT")
            nc.vector.tensor_copy(hT[:], pst[:])
            k = min(t // 12, NCHUNK - 1)
            col = (t - k * 12) * 128
            nc.sync.dma_start(
                out=aginc[k][:, col:col + rows], in_=hT[:, 0:rows])

        # chunked AllGather: issued from inside the B loop as soon as a
        # chunk's 12 tiles land, overlapping the collective with B and D
        def b_hook(t):
            if t in (11, 23, 35, 47, 48):
                k = min(t // 12, NCHUNK - 1)
                nc.gpsimd.collective_compute(
                    "AllGather", mybir.AluOpType.bypass,
                    replica_groups=[list(range(NCORES))],
                    ins=[aginc[k][:]], outs=[agoutc[k][:]])

        if "B" in phases:
            edge_phase(0, t0lo, t0hi, t0lo[:, 128:256], 8, NHID, T0W,
                       T0W, post0, None, hook=b_hook)

        # ---------------- phase D: layer-1 tables (chunk-major) -----------
        with ExitStack() as pd:
            xp1 = pd.enter_context(tc.tile_pool(name="pd_x", bufs=4))
            pp1 = pd.enter_context(
                tc.tile_pool(name="pd_ps", bufs=2, space="PSUM"))
            rp1 = pd.enter_context(tc.tile_pool(name="pd_row", bufs=2))
            dunits = ([(k, r) for k in range(NCHUNK) for r in range(NCORES)]
                      if "D" in phases else [])
            for k, r in dunits:
                base = r * NLOC + k * CCOLS
                w = CW[k]
                if w == CCOLS:
                    hx = xp1.tile([128, CCOLS], F16, tag="hx")
                    nc.sync.dma_start(
                        out=hx[:], in_=agoutc[k][bass.ds(sregs[r], 128), :])
                    row = rp1.tile([128, 12, 528], F16, tag="row")
                    for g2 in range(6):
                        # psf: each q's 512-col matmul exactly fills one
                        # PSUM bank (outputs must not cross 2KB banks)
                        psf = pp1.tile([128, 2, 512], F32, tag="psf")
                        psa = pp1.tile([128, 2, 16], F32, tag="psa")
                        for q in range(2):
                            hs = hx[:, (g2 * 2 + q) * 128:
                                    (g2 * 2 + q + 1) * 128]
                            nc.tensor.matmul(psf[:, q, :], hs, W1s[:],
                                             start=True, stop=True)
                            nc.tensor.matmul(psa[:, q, :], hs,
                                             W1as[:], start=True, stop=True)
                        nc.scalar.copy(row[:, 2 * g2:2 * g2 + 2, 0:264],
                                       psf[:, :, 0:264])
                        nc.vector.tensor_copy(
                            row[:, 2 * g2:2 * g2 + 2, 264:512],
                            psf[:, :, 264:512])
                        nc.vector.tensor_copy(
                            row[:, 2 * g2:2 * g2 + 2, 512:528],
                            psa[:, :, :])
                    eng = nc.scalar if r % 2 else nc.sync
                    if r < 4:
                        eng.dma_start(
                            out=t1lo[base:base + CCOLS, 0:528]
                            .rearrange("(g p) w -> p g w", p=128),
                            in_=row[:])
                    else:
                        o = base - SPLIT
                        eng.dma_start(
                            out=t1hi[o:o + CCOLS, 0:528]
                            .rearrange("(g p) w -> p g w", p=128),
                            in_=row[:])
                else:
                    hx = xp1.tile([128, w], F16, tag="hxt")
                    nc.sync.dma_start(
                        out=hx[:], in_=agoutc[k][bass.ds(sregs[r], 128), :])
                    psf = pp1.tile([128, 2, 512], F32, tag="psf")
                    psa = pp1.tile([128, 2, 16], F32, tag="psa")
                    nc.tensor.matmul(psf[0:w, 0, :], hx[:], W1s[:],
                                     start=True, stop=True)
                    nc.tensor.matmul(psa[0:w, 0, :], hx[:], W1as[:],
                                     start=True, stop=True)
                    row = rp1.tile([128, 12, 528], F16, tag="row")
                    nc.scalar.copy(row[0:w, 0, 0:264], psf[0:w, 0, 0:264])
                    nc.vector.tensor_copy(row[0:w, 0, 264:512],
                                          psf[0:w, 0, 264:512])
                    nc.vector.tensor_copy(row[0:w, 0, 512:528],
                                          psa[0:w, 0, :])
                    if r < 4:
                        nc.sync.dma_start(out=t1lo[base:base + w, 0:528],
                                          in_=row[0:w, 0, :])
                    else:
                        o = base - SPLIT
                        nc.sync.dma_start(out=t1hi[o:o + w, 0:528],
                                          in_=row[0:w, 0, :])

        # ---------------- phase E: layer-1 edges + epilogue ---------------
        def post1(t, P1, P2, op, pp2, fin):
            zbig, nmxb, seb = fin
            r8 = op.tile([128, 8], F32, tag="r8")
            nc.vector.tensor_scalar_add(r8[:], P2[:], 1e-16)
            nc.vector.reciprocal(r8[:], r8[:])
            nc.vector.tensor_scalar_mul(r8[:], r8[:], 1.0 / HEADS)
            zw = op.tile([128, 512], F32, tag="zw")
            nc.vector.tensor_tensor(
                out=zw[:].rearrange("p (d h) -> p d h", h=HEADS),
                in0=P1[:].rearrange("p (d h) -> p d h", h=HEADS),
                in1=r8[:].unsqueeze(1).broadcast_to([128, 64, HEADS]),
                op=mybir.AluOpType.mult)
            z = zbig[:, t * NCLASS:(t + 1) * NCLASS]
            nc.vector.reduce_sum(
                z, zw[:].rearrange("p (d h) -> p d h", h=HEADS),
                axis=mybir.AxisListType.X)
            nc.vector.tensor_tensor(out=z, in0=z, in1=b1s[:],
                                    op=mybir.AluOpType.add)
            nmx = nmxb[:, t:t + 1]
            nc.vector.reduce_max(nmx, z, axis=mybir.AxisListType.X,
                                 negate=True)
            ez = op.tile([128, NCLASS], F32, tag="ez")
            nc.scalar.activation(ez[:], z,
                                 mybir.ActivationFunctionType.Exp,
                                 bias=nmx, accum_out=seb[:, t:t + 1])

        if "E" in phases:
            fpool = stk.enter_context(tc.tile_pool(name="fin", bufs=1))
            zbig = fpool.tile([128, lt * NCLASS], F32)
            nmxb = fpool.tile([128, lt], F32)
            seb = fpool.tile([128, lt], F32)
            edge_phase(1, t1lo, t1hi, t1lo[:, 512:640], 8, 512, T1W,
                       T1W, post1, (zbig, nmxb, seb))
            # batched log-softmax tail: one Ln + two broadcast ops + 2 DMAs
            nc.scalar.activation(seb[:], seb[:],
                                 mybir.ActivationFunctionType.Ln)
            nc.vector.tensor_tensor(
                out=zbig[:].rearrange("p (t c) -> p t c", c=NCLASS),
                in0=zbig[:].rearrange("p (t c) -> p t c", c=NCLASS),
                in1=nmxb[:].unsqueeze(-1).broadcast_to([128, lt, NCLASS]),
                op=mybir.AluOpType.add)
            nc.vector.tensor_tensor(
                out=zbig[:].rearrange("p (t c) -> p t c", c=NCLASS),
                in0=zbig[:].rearrange("p (t c) -> p t c", c=NCLASS),
                in1=seb[:].unsqueeze(-1).broadcast_to([128, lt, NCLASS]),
                op=mybir.AluOpType.subtract)
            nfull = (lt - 1) * 128
            rlast = LAST_ROWS if lt == LT else 128
            nc.sync.dma_start(
                out=out[0:nfull, :].rearrange("(t p) c -> p t c", p=128),
                in_=zbig[:].rearrange("p (t c) -> p t c", c=NCLASS)
                [:, 0:lt - 1, :])
            nc.sync.dma_start(
                out=out[nfull:nfull + rlast, :],
                in_=zbig[0:rlast, (lt - 1) * NCLASS:lt * NCLASS])

    nc.compile()
    return nc


# --------------------------------------------------------------------------
# entry point
# --------------------------------------------------------------------------

def kernel(**inputs) -> np.ndarray:
    NLk, NHk, in_maps = _prep_inputs(**inputs)
    key = (NLk, NHk)
    if key not in _cache:
        _cache[key] = build(NLk, NHk)
    nc = _cache[key]
    res = run_bass_kernel_spmd(nc, in_maps, list(range(NCORES)))
    return np.concatenate([res.results[c]["out"] for c in range(NCORES)], 0)



# revision 6
# speedup vs baseline: 1.0099x; 1.0099x over previous
"""2-layer GAT (nn_GAT_31490700214331) on 8 Trainium2 NeuronCores.

Strategy (dst-sharded, SPMD, per-core-rotated node layout):
  - Nodes are block-partitioned: core c owns nodes [c*6250, (c+1)*6250).
  - Every table on core c uses a ROTATED row order: node n lives at row
    (n - c*6250) mod 50000, so each core's own nodes are rows 0..6249 and
    the single SPMD program has no core-dependent offsets.
  - Layer-0 features (h0 = x @ W0) + attention alphas are computed
    replicated on every core into a rotated f16 DRAM table; edges are
    grouped by dst tile (128 dsts), per-tile chunk counts specialized to
    the actual edge counts (max over cores), and source rows fetched with
    dma_gather through lo/hi table views (int16 indices < 32768).
  - Per-edge dst alphas come from one merged dma_gather over a 256B
    column window of the lo table.
  - Edge softmax (safe without segment-max: |e| <= ~5) and the weighted
    aggregation fuse into per-chunk 128x128 incidence matmuls in PSUM.
  - The ELU'd hidden state is AllGather'd in fp8(e3m4) chunks overlapped
    with phase B, rotated into per-core order, and layer 1 runs on an
    fp8 feature table (f16 alphas riding in the same 768B row) gathered
    at 768B/edge.
  - alpha projections fold into the weight matmuls on the host:
    h @ blockdiag(a) == x @ (W @ blockdiag(a)).

Self-contained: call kernel(**inputs) with the full-problem arrays.
"""
import numpy as np
from contextlib import ExitStack

import concourse.bacc as bacc
import concourse.bass as bass
import concourse.mybir as mybir
from concourse.tile import TileContext
from concourse.bass_utils import run_bass_kernel_spmd

F16 = mybir.dt.float16
F32 = mybir.dt.float32
F8 = mybir.dt.float8e3          # e3m4: 4 mantissa bits, max 15.5
I16 = mybir.dt.int16

N = 50000
NFEAT = 256
NHID = 128
NCLASS = 64
HEADS = 8
SLOPE = 0.2
NCORES = 8
NLOC = N // NCORES           # 6250
LT = (NLOC + 127) // 128     # 49 local dst tiles
LAST_ROWS = NLOC - (LT - 1) * 128   # 106 rows in the last tile
GT = 392                     # global node tiles (392*128 = 50176)
GROWS = GT * 128
SPLIT = 25000                # low/high gather-table split (4 core blocks)
CCOLS = 1536                 # collective chunk width (12 B-tiles)
NCHUNK = 5                   # 4 full chunks + 106-col tail
SENT = 300.0                 # dst_rel sentinel for padding slots
T0W = 256                    # t0 row: [h0(128)|as0(8)|ad0(8)|junk] f16
T1B = 768                    # t1 row bytes: [h1 f8(512)|as1,ad1 f16(32)|junk]

_cache = {}


# --------------------------------------------------------------------------
# host-side preparation
# --------------------------------------------------------------------------

def _wrap_idx(idx):
    """[n] int -> [128, n//16] int16 wrapped gather-index layout."""
    n = idx.shape[0]
    assert n % 16 == 0
    w = idx.reshape(n // 16, 16).T.astype(np.int16)
    return np.tile(w, (8, 1))


def _prep_edges(src, dst):
    cores = []
    for c in range(NCORES):
        m = (dst >= c * NLOC) & (dst < (c + 1) * NLOC)
        s = src[m].astype(np.int64)
        d = dst[m].astype(np.int64) - c * NLOC
        order = np.argsort(d, kind="stable")
        s, d = s[order], d[order]
        s_rot = (s - c * NLOC) % N
        tiles = []
        for t in range(LT):
            sel = (d >= t * 128) & (d < (t + 1) * 128)
            st, dt = s_rot[sel], d[sel] - t * 128
            lo = st < SPLIT
            tiles.append((st[lo], dt[lo], st[~lo] - SPLIT, dt[~lo]))
        cores.append(tiles)
    # per-tile chunk counts (max over cores so the SPMD program is shared)
    NLs, NHs = [], []
    for t in range(LT):
        nl = max(len(cores[c][t][0]) for c in range(NCORES))
        nh = max(len(cores[c][t][2]) for c in range(NCORES))
        NLs.append(max(1, (nl + 127) // 128))
        NHs.append(max(1, (nh + 127) // 128))
        assert NLs[t] * 128 <= 1024 and NHs[t] * 128 <= 1024

    out = []
    for c in range(NCORES):
        blocks = []
        for t in range(LT):
            NL, NH = NLs[t], NHs[t]
            CH = NL + NH
            sl, dl, sh, dh = cores[c][t]
            il = np.zeros(NL * 128, np.int64)
            il[: len(sl)] = sl
            ih = np.zeros(NH * 128, np.int64)
            ih[: len(sh)] = sh
            aa = np.zeros(CH * 128, np.int64)
            aa[: len(dl)] = t * 128 + dl
            aa[NL * 128: NL * 128 + len(dh)] = t * 128 + dh
            rl = np.full(NL * 128, SENT)
            rl[: len(dl)] = dl
            rh = np.full(NH * 128, SENT)
            rh[: len(dh)] = dh
            r = np.concatenate([rl, rh]).reshape(CH, 128).T
            drel = np.broadcast_to(
                r.astype(np.float16)[:, :, None],
                (128, CH, 8)).reshape(128, CH * 8)
            blocks.append(np.concatenate(
                [_wrap_idx(il), _wrap_idx(ih), _wrap_idx(aa),
                 np.ascontiguousarray(drel).view(np.int16)], axis=1))
        out.append(dict(epack=np.ascontiguousarray(
            np.concatenate(blocks, axis=1))))
    return NLs, NHs, out


def _prep_inputs(x, edge_index, W0, a_src0, a_dst0, b0, W1, a_src1, a_dst1,
                 b1):
    src = np.asarray(edge_index[0]).astype(np.int64)
    dst = np.asarray(edge_index[1]).astype(np.int64)
    NLs, NHs, edata = _prep_edges(src, dst)

    def bd(a):  # [H, D] -> blockdiag [H*D, H]
        a = np.asarray(a, np.float32)
        H, D = a.shape
        m = np.zeros((H * D, H), np.float32)
        for h in range(H):
            m[h * D:(h + 1) * D, h] = a[h]
        return m

    W0 = np.asarray(W0, np.float32)
    W1 = np.asarray(W1, np.float32)
    W0a = np.concatenate([W0 @ bd(a_src0), W0 @ bd(a_dst0)], 1)  # [256, 16]
    # head-innermost feature interleave: new col d*8+h <- old col h*D+d
    perm0 = np.array([(f % 8) * 16 + f // 8 for f in range(128)])
    perm1 = np.array([(f % 8) * 64 + f // 8 for f in range(512)])
    W0cat = np.concatenate([W0[:, perm0], W0a], 1)               # [256, 144]
    W1a = np.concatenate([W1 @ bd(a_src1), W1 @ bd(a_dst1)], 1)  # [128, 16]

    x = np.asarray(x, np.float32)
    ident = np.eye(128, dtype=np.float16)
    colio = np.tile(np.arange(128, dtype=np.float16)[None, :], (128, 1))
    b0b = np.tile(np.asarray(b0, np.float32)[None, :], (128, 1))
    b1b = np.tile(np.asarray(b1, np.float32)[None, :], (128, 1))

    in_maps = []
    for c in range(NCORES):
        rot = np.roll(np.arange(N), -c * NLOC)
        xr = np.zeros((GROWS, NFEAT), np.float16)
        xr[:N] = x[rot].astype(np.float16)
        # [gg, 128(j feat), 2(g), 2(k), 128(p node)]: partition = feature,
        # per-partition contiguous 1KB runs
        xtt = (xr.reshape(GROWS // 256, 2, 128, 2, 128)
               .transpose(0, 4, 1, 3, 2))
        m = dict(
            xT=np.ascontiguousarray(xtt),
            W0=np.ascontiguousarray(
                W0cat.astype(np.float16).reshape(2, 128, NHID + 16)),
            W1=np.ascontiguousarray(W1[perm0][:, perm1].astype(np.float16)),
            W1a=np.ascontiguousarray(W1a[perm0].astype(np.float16)),
            b0b=np.ascontiguousarray(b0b[:, perm0]), b1b=b1b,
            ident=ident, colio=colio,
            **edata[c],
        )
        in_maps.append(m)
    return NLs, NHs, in_maps


# --------------------------------------------------------------------------
# device program
# --------------------------------------------------------------------------

def build(NLs, NHs, lt=LT, gt=GT, debug=False, phases="ABCDE",
          sim_safe=False):
    NLs, NHs = list(NLs), list(NHs)
    HID16 = NHID + 16
    EPW = [(NLs[t] + NHs[t]) * 24 for t in range(lt)]
    EOFF = np.concatenate([[0], np.cumsum(EPW)]).astype(int)
    nc = bacc.Bacc("TRN2")
    xT = nc.dram_tensor("xT", [GROWS // 256, 128, 2, 2, 128], F16,
                        kind="ExternalInput")
    W0i = nc.dram_tensor("W0", [2, 128, NHID + 16], F16,
                         kind="ExternalInput")
    W1i = nc.dram_tensor("W1", [NHID, 512], F16, kind="ExternalInput")
    W1ai = nc.dram_tensor("W1a", [NHID, 16], F16, kind="ExternalInput")
    b0bi = nc.dram_tensor("b0b", [128, NHID], F32, kind="ExternalInput")
    b1bi = nc.dram_tensor("b1b", [128, NCLASS], F32, kind="ExternalInput")
    identi = nc.dram_tensor("ident", [128, 128], F16, kind="ExternalInput")
    colioi = nc.dram_tensor("colio", [128, 128], F16, kind="ExternalInput")
    epacki = nc.dram_tensor("epack", [128, int(EOFF[-1])], I16,
                            kind="ExternalInput")
    out = nc.dram_tensor("out", [NLOC, NCLASS], F32, kind="ExternalOutput")

    with TileContext(nc) as tc, ExitStack() as stk:
        regs = {}

        def reg_of(n):
            if n not in regs:
                regs[n] = nc.gpsimd.to_reg(n)
            return regs[n]

        dpool = stk.enter_context(
            tc.tile_pool(name="dram", bufs=1, space="DRAM"))
        t0lo = dpool.tile([SPLIT, T0W], F16, tag="t0lo")
        t0hi = dpool.tile([GROWS - SPLIT, T0W], F16, tag="t0hi")
        t1lo = dpool.tile([SPLIT, T1B], F8, tag="t1lo")
        t1hi = dpool.tile([GROWS - SPLIT, T1B], F8, tag="t1hi")
        CW = [CCOLS] * 4 + [NLOC - 4 * CCOLS]   # 4x1536 + 106
        aginc = [dpool.tile([128, CW[k]], F8, tag=f"agin{k}",
                            name=f"agin{k}")
                 for k in range(NCHUNK)]
        agoutc = [dpool.tile([NCORES * 128, CW[k]], F8, tag=f"agout{k}",
                             addr_space="Shared", name=f"agout{k}")
                  for k in range(NCHUNK)]

        cpool = stk.enter_context(tc.tile_pool(name="const", bufs=1))
        W0s = cpool.tile([128, 2, NHID + 16], F16)
        nc.sync.dma_start(out=W0s[:], in_=W0i.rearrange("k p n -> p k n"))
        W1s = cpool.tile([128, 512], F16)
        nc.sync.dma_start(out=W1s[:], in_=W1i[:])
        W1as = cpool.tile([128, 16], F16)
        nc.sync.dma_start(out=W1as[:], in_=W1ai[:])
        b0s = cpool.tile([128, NHID], F32)
        nc.sync.dma_start(out=b0s[:], in_=b0bi[:])
        b1s = cpool.tile([128, NCLASS], F32)
        nc.sync.dma_start(out=b1s[:], in_=b1bi[:])
        idents = cpool.tile([128, 128], F16)
        nc.sync.dma_start(out=idents[:], in_=identi[:])
        colios = cpool.tile([128, 128], F16)
        nc.sync.dma_start(out=colios[:], in_=colioi[:])

        pid = nc.partition_id(engines=[mybir.EngineType.SP])
        sregs = [nc.sync.snap(((j + pid) % NCORES) * 128)
                 for j in range(NCORES)]

        # ---------------- phase A: layer-0 tables (replicated) ------------
        with ExitStack() as pa:
            xp = pa.enter_context(tc.tile_pool(name="pa_x", bufs=4))
            pp = pa.enter_context(
                tc.tile_pool(name="pa_ps", bufs=2, space="PSUM"))
            rp = pa.enter_context(tc.tile_pool(name="pa_row", bufs=4))
            assert gt % 4 == 0
            for gq in range(gt // 4):
                # two 2-group units per load to halve DMA issue count
                xa = xp.tile([128, 2, 2, 2, 128], F16, tag="xa")
                leng = nc.sync if gq % 2 else nc.gpsimd
                leng.dma_start(
                    out=xa[:],
                    in_=xT[2 * gq:2 * gq + 2]
                    .rearrange("G p g k f -> p G g k f"))
                for G in range(2):
                    gg = 2 * gq + G
                    row = rp.tile([128, 2, T0W], F16, tag="row")
                    for g2 in range(2):
                        ps = pp.tile([128, HID16], F32, tag=f"ps{g2}")
                        for k in range(2):
                            nc.tensor.matmul(ps[:], xa[:, G, g2, k, :],
                                             W0s[:, k, :],
                                             start=(k == 0), stop=(k == 1))
                        eng2 = nc.vector.tensor_copy if g2 else nc.scalar.copy
                        eng2(row[:, g2, 0:HID16], ps[:])
                    eng = nc.scalar if gg % 2 else nc.sync
                    g0 = 2 * gg * 128
                    if g0 + 256 <= SPLIT:
                        eng.dma_start(
                            out=t0lo[g0:g0 + 256, 0:HID16]
                            .rearrange("(g p) w -> p g w", p=128),
                            in_=row[:, :, 0:HID16])
                    elif g0 >= SPLIT:
                        o = g0 - SPLIT
                        eng.dma_start(
                            out=t0hi[o:o + 256, 0:HID16]
                            .rearrange("(g p) w -> p g w", p=128),
                            in_=row[:, :, 0:HID16])
                    else:
                        # group straddles the lo/hi split inside tile g2=1
                        cut = SPLIT - g0 - 128
                        eng.dma_start(out=t0lo[g0:g0 + 128, 0:HID16],
                                      in_=row[:, 0, 0:HID16])
                        eng.dma_start(
                            out=t0lo[g0 + 128:SPLIT, 0:HID16],
                            in_=row[0:cut, 1, 0:HID16])
                        eng.dma_start(
                            out=t0hi[0:256 - 128 - cut, 0:HID16],
                            in_=row[cut:128, 1, 0:HID16])

        # ---------------- shared edge phase -------------------------------
        def edge_phase(layer, post_fn, fin, hook=None):
            if layer == 0:
                tbl_lo, tbl_hi, trow, fdim = t0lo, t0hi, T0W, NHID
                gdt, adt, awcols = F16, F16, 128
                awin = t0lo[:, 128:256]
            else:
                tbl_lo, tbl_hi, trow, fdim = t1lo, t1hi, T1B, 512
                gdt, adt, awcols = F8, F8, 256
                awin = t1lo[:, 512:768]
            D = fdim // HEADS
            with ExitStack() as pb:
                ip = pb.enter_context(
                    tc.tile_pool(name=f"ix{layer}", bufs=4))
                gp = pb.enter_context(
                    tc.tile_pool(name=f"gg{layer}", bufs=4))
                apl = pb.enter_context(
                    tc.tile_pool(name=f"ga{layer}", bufs=3))
                rp2 = pb.enter_context(
                    tc.tile_pool(name=f"rh{layer}", bufs=3))
                pp2 = pb.enter_context(
                    tc.tile_pool(name=f"ps{layer}", bufs=2, space="PSUM"))
                op = pb.enter_context(
                    tc.tile_pool(name=f"po{layer}", bufs=3))
                for t in range(lt):
                    NL, NH = NLs[t], NHs[t]
                    CH = NL + NH
                    NLI, NHI = NL * 128, NH * 128
                    o_ih = NL * 8
                    o_ea = CH * 8
                    o_dr = CH * 16
                    ep = ip.tile([128, EPW[t]], I16, tag="ep")
                    nc.sync.dma_start(
                        out=ep[:], in_=epacki[:, EOFF[t]:EOFF[t + 1]])
                    il = ep[:, 0:NL * 8]
                    ih = ep[:, o_ih:o_ih + NH * 8]
                    ea = ep[:, o_ea:o_ea + CH * 8]
                    dr8 = (ep[:, o_dr:o_dr + CH * 8].bitcast(F16)
                           .rearrange("p (c e) -> p c e", e=8))

                    G = gp.tile([128, CH, trow], gdt, tag="G")
                    nc.gpsimd.dma_gather(G[:, 0:NL, :], tbl_lo[:], il,
                                         NLI, reg_of(NLI), trow,
                                         elem_step=trow)
                    nc.gpsimd.dma_gather(G[:, NL:CH, :], tbl_hi[:],
                                         ih, NHI, reg_of(NHI), trow,
                                         elem_step=trow)
                    # dst-alpha gathers: 256B column window of the lo table
                    # (<=1024 idx per SWDGE call)
                    A = apl.tile([128, CH, awcols], adt, tag="A")
                    nc.gpsimd.dma_gather(A[:, 0:NL, :], awin,
                                         ea[:, 0:NL * 8], NLI, reg_of(NLI),
                                         awcols, elem_step=trow)
                    nc.gpsimd.dma_gather(A[:, NL:CH, :], awin,
                                         ea[:, NL * 8:CH * 8], NHI,
                                         reg_of(NHI), awcols, elem_step=trow)
                    if layer == 0:
                        g_as = G[:, :, fdim:fdim + 8]
                        a_ad = A[:, :, 8:16]
                        g_f = G[:, :, 0:fdim]
                    else:
                        g_as = G[:, :, 512:528].bitcast(F16)
                        a_ad = A[:, :, 16:32].bitcast(F16)
                        g_f = G[:, :, 0:fdim]

                    inc = rp2.tile([128, CH, 128], F16, tag="inc")
                    nc.vector.tensor_tensor(
                        out=inc[:].rearrange("p c (g e) -> p c g e", e=8),
                        in0=dr8.unsqueeze(2)
                        .broadcast_to([128, CH, 16, 8]),
                        in1=colios[:].rearrange("p (g e) -> p g e", e=8)
                        .unsqueeze(1).broadcast_to([128, CH, 16, 8]),
                        op=mybir.AluOpType.is_equal)
                    EX = rp2.tile([128, CH, 8], F16, tag="EX")
                    nc.vector.tensor_tensor(
                        out=EX[:], in0=g_as, in1=a_ad,
                        op=mybir.AluOpType.add)
                    if sim_safe:
                        EXr = rp2.tile([128, CH, 8], F16, tag="EXr")
                        nc.scalar.activation(
                            EXr[:], EX[:],
                            mybir.ActivationFunctionType.Relu, scale=0.8)
                        nc.vector.tensor_scalar_mul(EX[:], EX[:], SLOPE)
                        nc.vector.tensor_tensor(
                            out=EX[:], in0=EX[:], in1=EXr[:],
                            op=mybir.AluOpType.add)
                    else:
                        nc.scalar.activation(
                            EX[:], EX[:],
                            mybir.ActivationFunctionType.Prelu, alpha=SLOPE)
                    nc.scalar.activation(
                        EX[:], EX[:], mybir.ActivationFunctionType.Exp)

                    R = rp2.tile([128, CH, fdim], F16, tag="R")
                    nc.vector.tensor_tensor(
                        out=R[:].rearrange("p c (d h) -> p c d h", h=HEADS),
                        in0=g_f.rearrange("p c (d h) -> p c d h", h=HEADS),
                        in1=EX[:].unsqueeze(2)
                        .broadcast_to([128, CH, D, HEADS]),
                        op=mybir.AluOpType.mult)

                    P1 = pp2.tile([128, fdim], F32, tag="P1")
                    P2 = pp2.tile([128, 8], F32, tag="P2")
                    for ch in range(CH):
                        nc.tensor.matmul(P1[:], inc[:, ch, :],
                                         R[:, ch, 0:fdim],
                                         start=(ch == 0),
                                         stop=(ch == CH - 1))
                    for ch in range(CH):
                        nc.tensor.matmul(P2[:], inc[:, ch, :],
                                         EX[:, ch, :],
                                         start=(ch == 0),
                                         stop=(ch == CH - 1))
                    post_fn(t, P1, P2, op, pp2, fin)
                    if hook is not None:
                        hook(t)

        # ---- L0 post: softmax-div, +b0, ELU, transpose, store ------------
        def post0(t, P1, P2, op, pp2, fin):
            rows = 128 if t < lt - 1 else LAST_ROWS
            r8 = op.tile([128, 8], F32, tag="r8")
            nc.vector.tensor_scalar_add(r8[:], P2[:], 1e-16)
            nc.vector.reciprocal(r8[:], r8[:])
            z = op.tile([128, NHID], F32, tag="z")
            nc.vector.tensor_tensor(
                out=z[:].rearrange("p (d h) -> p d h", h=HEADS),
                in0=P1[:].rearrange("p (d h) -> p d h", h=HEADS),
                in1=r8[:].unsqueeze(1).broadcast_to([128, 16, HEADS]),
                op=mybir.AluOpType.mult)
            nc.vector.tensor_tensor(out=z[:], in0=z[:], in1=b0s[:],
                                    op=mybir.AluOpType.add)
            zm = op.tile([128, NHID], F32, tag="zm")
            nc.vector.tensor_scalar_min(zm[:], z[:], 0.0)
            nc.scalar.activation(zm[:], zm[:],
                                 mybir.ActivationFunctionType.Exp)
            zp = op.tile([128, NHID], F32, tag="zp")
            nc.vector.tensor_scalar_max(zp[:], z[:], 0.0)
            nc.vector.tensor_tensor(out=zp[:], in0=zp[:], in1=zm[:],
                                    op=mybir.AluOpType.add)
            h1 = op.tile([128, NHID], F16, tag="h1")
            nc.vector.tensor_scalar_add(h1[:], zp[:], -1.0)
            pst = pp2.tile([128, 128], F16, tag="pst")
            nc.tensor.transpose(pst[:], h1[:], idents[:])
            hT = op.tile([128, 128], F8, tag="hT")
            nc.vector.tensor_copy(hT[:], pst[:])
            k = min(t // 12, NCHUNK - 1)
            col = (t - k * 12) * 128
            nc.sync.dma_start(
                out=aginc[k][:, col:col + rows], in_=hT[:, 0:rows])

        # chunked AllGather: issued from inside the B loop as soon as a
        # chunk's 12 tiles land, overlapping the collective with B and D
        def b_hook(t):
            if t in (11, 23, 35, 47, 48):
                k = min(t // 12, NCHUNK - 1)
                nc.gpsimd.collective_compute(
                    "AllGather", mybir.AluOpType.bypass,
                    replica_groups=[list(range(NCORES))],
                    ins=[aginc[k][:]], outs=[agoutc[k][:]])

        if "B" in phases:
            edge_phase(0, post0, None, hook=b_hook)

        # ---------------- phase D: layer-1 tables (chunk-major) -----------
        with ExitStack() as pd:
            xp1 = pd.enter_context(tc.tile_pool(name="pd_x", bufs=4))
            pp1 = pd.enter_context(
                tc.tile_pool(name="pd_ps", bufs=2, space="PSUM"))
            rp1 = pd.enter_context(tc.tile_pool(name="pd_row", bufs=2))
            dunits = ([(k, r) for k in range(NCHUNK) for r in range(NCORES)]
                      if "D" in phases else [])
            wengs = [nc.sync, nc.scalar, nc.gpsimd]
            for k, r in dunits:
                base = r * NLOC + k * CCOLS
                w = CW[k]
                if w == CCOLS:
                    hx8 = xp1.tile([128, CCOLS], F8, tag="hx8")
                    nc.sync.dma_start(
                        out=hx8[:],
                        in_=agoutc[k][bass.ds(sregs[r], 128), :])
                    hx = xp1.tile([128, CCOLS], F16, tag="hx")
                    nc.gpsimd.tensor_copy(hx[:], hx8[:])
                    row = rp1.tile([128, 12, T1B], F8, tag="row")
                    ralp = row[:, :, 512:544].bitcast(F16)
                    for g2 in range(6):
                        # each q's 512-col matmul exactly fills one PSUM
                        # bank (outputs must not cross 2KB banks)
                        psf = pp1.tile([128, 2, 512], F32, tag="psf")
                        psa = pp1.tile([128, 2, 16], F32, tag="psa")
                        for q in range(2):
                            hs = hx[:, (g2 * 2 + q) * 128:
                                    (g2 * 2 + q + 1) * 128]
                            nc.tensor.matmul(psf[:, q, :], hs, W1s[:],
                                             start=True, stop=True)
                            nc.tensor.matmul(psa[:, q, :], hs,
                                             W1as[:], start=True, stop=True)
                        nc.scalar.copy(row[:, 2 * g2:2 * g2 + 2, 0:256],
                                       psf[:, :, 0:256])
                        nc.vector.tensor_copy(
                            row[:, 2 * g2:2 * g2 + 2, 256:512],
                            psf[:, :, 256:512])
                        nc.vector.tensor_copy(
                            ralp[:, 2 * g2:2 * g2 + 2, :], psa[:, :, :])
                    eng = wengs[(k * NCORES + r) % 3]
                    if r < 4:
                        eng.dma_start(
                            out=t1lo[base:base + CCOLS, 0:544]
                            .rearrange("(g p) w -> p g w", p=128),
                            in_=row[:, :, 0:544])
                    else:
                        o = base - SPLIT
                        eng.dma_start(
                            out=t1hi[o:o + CCOLS, 0:544]
                            .rearrange("(g p) w -> p g w", p=128),
                            in_=row[:, :, 0:544])
                else:
                    hx8 = xp1.tile([128, w], F8, tag="hx8t")
                    nc.sync.dma_start(
                        out=hx8[:],
                        in_=agoutc[k][bass.ds(sregs[r], 128), :])
                    hx = xp1.tile([128, w], F16, tag="hxt")
                    nc.gpsimd.tensor_copy(hx[:], hx8[:])
                    psf = pp1.tile([128, 2, 512], F32, tag="psf")
                    psa = pp1.tile([128, 2, 16], F32, tag="psa")
                    nc.tensor.matmul(psf[0:w, 0, :], hx[:], W1s[:],
                                     start=True, stop=True)
                    nc.tensor.matmul(psa[0:w, 0, :], hx[:], W1as[:],
                                     start=True, stop=True)
                    row = rp1.tile([128, 12, T1B], F8, tag="row")
                    ralp = row[:, :, 512:544].bitcast(F16)
                    nc.scalar.copy(row[0:w, 0, 0:256], psf[0:w, 0, 0:256])
                    nc.vector.tensor_copy(row[0:w, 0, 256:512],
                                          psf[0:w, 0, 256:512])
                    nc.vector.tensor_copy(ralp[0:w, 0, :], psa[0:w, 0, :])
                    if r < 4:
                        nc.sync.dma_start(out=t1lo[base:base + w, 0:544],
                                          in_=row[0:w, 0, 0:544])
                    else:
                        o = base - SPLIT
                        nc.sync.dma_start(out=t1hi[o:o + w, 0:544],
                                          in_=row[0:w, 0, 0:544])

        # ---------------- phase E: layer-1 edges + epilogue ---------------
        def post1(t, P1, P2, op, pp2, fin):
            zbig, nmxb, seb = fin
            r8 = op.tile([128, 8], F32, tag="r8")
            nc.vector.tensor_scalar_add(r8[:], P2[:], 1e-16)
            nc.vector.reciprocal(r8[:], r8[:])
            nc.vector.tensor_scalar_mul(r8[:], r8[:], 1.0 / HEADS)
            zw = op.tile([128, 512], F32, tag="zw")
            nc.vector.tensor_tensor(
                out=zw[:].rearrange("p (d h) -> p d h", h=HEADS),
                in0=P1[:].rearrange("p (d h) -> p d h", h=HEADS),
                in1=r8[:].unsqueeze(1).broadcast_to([128, 64, HEADS]),
                op=mybir.AluOpType.mult)
            z = zbig[:, t * NCLASS:(t + 1) * NCLASS]
            nc.vector.reduce_sum(
                z, zw[:].rearrange("p (d h) -> p d h", h=HEADS),
                axis=mybir.AxisListType.X)
            nc.vector.tensor_tensor(out=z, in0=z, in1=b1s[:],
                                    op=mybir.AluOpType.add)
            nmx = nmxb[:, t:t + 1]
            nc.vector.reduce_max(nmx, z, axis=mybir.AxisListType.X,
                                 negate=True)
            ez = op.tile([128, NCLASS], F32, tag="ez")
            nc.scalar.activation(ez[:], z,
                                 mybir.ActivationFunctionType.Exp,
                                 bias=nmx, accum_out=seb[:, t:t + 1])

        if "E" in phases:
            fpool = stk.enter_context(tc.tile_pool(name="fin", bufs=1))
            zbig = fpool.tile([128, lt * NCLASS], F32)
            nmxb = fpool.tile([128, lt], F32)
            seb = fpool.tile([128, lt], F32)
            edge_phase(1, post1, (zbig, nmxb, seb))
            # batched log-softmax tail: one Ln + two broadcast ops + 2 DMAs
            nc.scalar.activation(seb[:], seb[:],
                                 mybir.ActivationFunctionType.Ln)
            nc.vector.tensor_tensor(
                out=zbig[:].rearrange("p (t c) -> p t c", c=NCLASS),
                in0=zbig[:].rearrange("p (t c) -> p t c", c=NCLASS),
                in1=nmxb[:].unsqueeze(-1).broadcast_to([128, lt, NCLASS]),
                op=mybir.AluOpType.add)
            nc.vector.tensor_tensor(
                out=zbig[:].rearrange("p (t c) -> p t c", c=NCLASS),
                in0=zbig[:].rearrange("p (t c) -> p t c", c=NCLASS),
                in1=seb[:].unsqueeze(-1).broadcast_to([128, lt, NCLASS]),
                op=mybir.AluOpType.subtract)
            nfull = (lt - 1) * 128
            rlast = LAST_ROWS if lt == LT else 128
            nc.sync.dma_start(
                out=out[0:nfull, :].rearrange("(t p) c -> p t c", p=128),
                in_=zbig[:].rearrange("p (t c) -> p t c", c=NCLASS)
                [:, 0:lt - 1, :])
            nc.sync.dma_start(
                out=out[nfull:nfull + rlast, :],
                in_=zbig[0:rlast, (lt - 1) * NCLASS:lt * NCLASS])

    nc.compile()
    return nc


# --------------------------------------------------------------------------
# entry point
# --------------------------------------------------------------------------

def kernel(**inputs) -> np.ndarray:
    NLs, NHs, in_maps = _prep_inputs(**inputs)
    key = (tuple(NLs), tuple(NHs))
    if key not in _cache:
        _cache[key] = build(NLs, NHs)
    nc = _cache[key]
    res = run_bass_kernel_spmd(nc, in_maps, list(range(NCORES)))
    return np.concatenate([res.results[c]["out"] for c in range(NCORES)], 0)


# revision 8
# speedup vs baseline: 1.1165x; 1.1055x over previous
"""2-layer GAT (nn_GAT_31490700214331) on 8 Trainium2 NeuronCores.

Strategy (dst-sharded, SPMD, per-core-rotated node layout):
  - Nodes are block-partitioned: core c owns nodes [c*6250, (c+1)*6250).
  - Every table on core c uses a ROTATED row order: node n lives at row
    (n - c*6250) mod 50000, so each core's own nodes are rows 0..6249 and
    the single SPMD program has no core-dependent offsets.
  - Layer-0 features (h0 = x @ W0) + attention alphas are computed
    replicated on every core into a rotated f16 DRAM table; edges are
    grouped by dst tile (128 dsts), per-tile chunk counts specialized to
    the actual edge counts (max over cores), and source rows fetched with
    dma_gather through lo/hi table views (int16 indices < 32768).
  - Per-edge dst alphas come from one merged dma_gather over a 256B
    column window of the lo table.
  - Edge softmax (safe without segment-max: |e| <= ~5) and the weighted
    aggregation fuse into per-chunk 128x128 incidence matmuls in PSUM.
  - The ELU'd hidden state is AllGather'd in fp8(e3m4) chunks overlapped
    with phase B, rotated into per-core order, and layer 1 runs on an
    fp8 feature table (f16 alphas riding in the same 768B row) gathered
    at 768B/edge.
  - alpha projections fold into the weight matmuls on the host:
    h @ blockdiag(a) == x @ (W @ blockdiag(a)).

Self-contained: call kernel(**inputs) with the full-problem arrays.
"""
import numpy as np
from contextlib import ExitStack

import concourse.bacc as bacc
import concourse.bass as bass
import concourse.mybir as mybir
from concourse.tile import TileContext
from concourse.bass_utils import run_bass_kernel_spmd

F16 = mybir.dt.float16
F32 = mybir.dt.float32
F8 = mybir.dt.float8e3          # e3m4: 4 mantissa bits, max 15.5
I16 = mybir.dt.int16

N = 50000
NFEAT = 256
NHID = 128
NCLASS = 64
HEADS = 8
SLOPE = 0.2
NCORES = 8
NLOC = N // NCORES           # 6250
LT = (NLOC + 127) // 128     # 49 local dst tiles
LAST_ROWS = NLOC - (LT - 1) * 128   # 106 rows in the last tile
GT = 392                     # global node tiles (392*128 = 50176)
GROWS = GT * 128
SPLIT = 25000                # low/high gather-table split (4 core blocks)
CCOLS = 1536                 # collective chunk width (12 B-tiles)
NCHUNK = 5                   # 4 full chunks + 106-col tail
SENT = 300.0                 # dst_rel sentinel for padding slots
T0W = 256                    # t0 row: [h0(128)|as0(8)|ad0(8)|junk] f16
T1W = 640                    # t1 row: [h1(512)|as1(8)|ad1(8)|junk] f16

_cache = {}


# --------------------------------------------------------------------------
# host-side preparation
# --------------------------------------------------------------------------

def _wrap_idx(idx):
    """[n] int -> [128, n//16] int16 wrapped gather-index layout."""
    n = idx.shape[0]
    assert n % 16 == 0
    w = idx.reshape(n // 16, 16).T.astype(np.int16)
    return np.tile(w, (8, 1))


def _prep_edges(src, dst):
    cores = []
    for c in range(NCORES):
        m = (dst >= c * NLOC) & (dst < (c + 1) * NLOC)
        s = src[m].astype(np.int64)
        d = dst[m].astype(np.int64) - c * NLOC
        order = np.argsort(d, kind="stable")
        s, d = s[order], d[order]
        s_rot = (s - c * NLOC) % N
        tiles = []
        for t in range(LT):
            sel = (d >= t * 128) & (d < (t + 1) * 128)
            st, dt = s_rot[sel], d[sel] - t * 128
            lo = st < SPLIT
            tiles.append((st[lo], dt[lo], st[~lo] - SPLIT, dt[~lo]))
        cores.append(tiles)
    # per-tile chunk counts (max over cores so the SPMD program is shared)
    NLs, NHs = [], []
    for t in range(LT):
        nl = max(len(cores[c][t][0]) for c in range(NCORES))
        nh = max(len(cores[c][t][2]) for c in range(NCORES))
        NLs.append(max(1, (nl + 127) // 128))
        NHs.append(max(1, (nh + 127) // 128))
        assert NLs[t] * 128 <= 1024 and NHs[t] * 128 <= 1024

    out = []
    for c in range(NCORES):
        blocks = []
        for t in range(LT):
            NL, NH = NLs[t], NHs[t]
            CH = NL + NH
            sl, dl, sh, dh = cores[c][t]
            il = np.zeros(NL * 128, np.int64)
            il[: len(sl)] = sl
            ih = np.zeros(NH * 128, np.int64)
            ih[: len(sh)] = sh
            aa = np.zeros(CH * 128, np.int64)
            aa[: len(dl)] = t * 128 + dl
            aa[NL * 128: NL * 128 + len(dh)] = t * 128 + dh
            rl = np.full(NL * 128, SENT)
            rl[: len(dl)] = dl
            rh = np.full(NH * 128, SENT)
            rh[: len(dh)] = dh
            r = np.concatenate([rl, rh]).reshape(CH, 128).T
            drel = np.broadcast_to(
                r.astype(np.float16)[:, :, None],
                (128, CH, 8)).reshape(128, CH * 8)
            blocks.append(np.concatenate(
                [_wrap_idx(il), _wrap_idx(ih), _wrap_idx(aa),
                 np.ascontiguousarray(drel).view(np.int16)], axis=1))
        out.append(dict(epack=np.ascontiguousarray(
            np.concatenate(blocks, axis=1))))
    return NLs, NHs, out


def _prep_inputs(x, edge_index, W0, a_src0, a_dst0, b0, W1, a_src1, a_dst1,
                 b1):
    src = np.asarray(edge_index[0]).astype(np.int64)
    dst = np.asarray(edge_index[1]).astype(np.int64)
    NLs, NHs, edata = _prep_edges(src, dst)

    def bd(a):  # [H, D] -> blockdiag [H*D, H]
        a = np.asarray(a, np.float32)
        H, D = a.shape
        m = np.zeros((H * D, H), np.float32)
        for h in range(H):
            m[h * D:(h + 1) * D, h] = a[h]
        return m

    W0 = np.asarray(W0, np.float32)
    W1 = np.asarray(W1, np.float32)
    W0a = np.concatenate([W0 @ bd(a_src0), W0 @ bd(a_dst0)], 1)  # [256, 16]
    # head-innermost feature interleave: new col d*8+h <- old col h*D+d
    perm0 = np.array([(f % 8) * 16 + f // 8 for f in range(128)])
    perm1 = np.array([(f % 8) * 64 + f // 8 for f in range(512)])
    W0cat = np.concatenate([W0[:, perm0], W0a], 1)               # [256, 144]
    W1a = np.concatenate([W1 @ bd(a_src1), W1 @ bd(a_dst1)], 1)  # [128, 16]

    x = np.asarray(x, np.float32)
    ident = np.eye(128, dtype=np.float16)
    colio = np.tile(np.arange(128, dtype=np.float16)[None, :], (128, 1))
    b0b = np.tile(np.asarray(b0, np.float32)[None, :], (128, 1))
    b1b = np.tile(np.asarray(b1, np.float32)[None, :], (128, 1))

    in_maps = []
    for c in range(NCORES):
        rot = np.roll(np.arange(N), -c * NLOC)
        xr = np.zeros((GROWS, NFEAT), np.float16)
        xr[:N] = x[rot].astype(np.float16)
        # [gg, 128(j feat), 2(g), 2(k), 128(p node)]: partition = feature,
        # per-partition contiguous 1KB runs
        xtt = (xr.reshape(GROWS // 256, 2, 128, 2, 128)
               .transpose(0, 4, 1, 3, 2))
        m = dict(
            xT=np.ascontiguousarray(xtt),
            W0=np.ascontiguousarray(
                W0cat.astype(np.float16).reshape(2, 128, NHID + 16)),
            W1=np.ascontiguousarray(W1[perm0][:, perm1].astype(np.float16)),
            W1a=np.ascontiguousarray(W1a[perm0].astype(np.float16)),
            b0b=np.ascontiguousarray(b0b[:, perm0]), b1b=b1b,
            ident=ident, colio=colio,
            **edata[c],
        )
        in_maps.append(m)
    return NLs, NHs, in_maps


# --------------------------------------------------------------------------
# device program
# --------------------------------------------------------------------------

def build(NLs, NHs, lt=LT, gt=GT, debug=False, phases="ABCDE",
          sim_safe=False):
    NLs, NHs = list(NLs), list(NHs)
    HID16 = NHID + 16
    EPW = [(NLs[t] + NHs[t]) * 24 for t in range(lt)]
    EOFF = np.concatenate([[0], np.cumsum(EPW)]).astype(int)
    nc = bacc.Bacc("TRN2")
    xT = nc.dram_tensor("xT", [GROWS // 256, 128, 2, 2, 128], F16,
                        kind="ExternalInput")
    W0i = nc.dram_tensor("W0", [2, 128, NHID + 16], F16,
                         kind="ExternalInput")
    W1i = nc.dram_tensor("W1", [NHID, 512], F16, kind="ExternalInput")
    W1ai = nc.dram_tensor("W1a", [NHID, 16], F16, kind="ExternalInput")
    b0bi = nc.dram_tensor("b0b", [128, NHID], F32, kind="ExternalInput")
    b1bi = nc.dram_tensor("b1b", [128, NCLASS], F32, kind="ExternalInput")
    identi = nc.dram_tensor("ident", [128, 128], F16, kind="ExternalInput")
    colioi = nc.dram_tensor("colio", [128, 128], F16, kind="ExternalInput")
    epacki = nc.dram_tensor("epack", [128, int(EOFF[-1])], I16,
                            kind="ExternalInput")
    out = nc.dram_tensor("out", [NLOC, NCLASS], F32, kind="ExternalOutput")

    with TileContext(nc) as tc, ExitStack() as stk:
        regs = {}

        def reg_of(n):
            if n not in regs:
                regs[n] = nc.gpsimd.to_reg(n)
            return regs[n]

        dpool = stk.enter_context(
            tc.tile_pool(name="dram", bufs=1, space="DRAM"))
        t0lo = dpool.tile([SPLIT, T0W], F16, tag="t0lo")
        t0hi = dpool.tile([GROWS - SPLIT, T0W], F16, tag="t0hi")
        t1lo = dpool.tile([SPLIT, T1W], F16, tag="t1lo")
        t1hi = dpool.tile([GROWS - SPLIT, T1W], F16, tag="t1hi")
        CW = [CCOLS] * 4 + [NLOC - 4 * CCOLS]   # 4x1536 + 106
        aginc = [dpool.tile([128, CW[k]], F8, tag=f"agin{k}",
                            name=f"agin{k}")
                 for k in range(NCHUNK)]
        agoutc = [dpool.tile([NCORES * 128, CW[k]], F8, tag=f"agout{k}",
                             addr_space="Shared", name=f"agout{k}")
                  for k in range(NCHUNK)]

        cpool = stk.enter_context(tc.tile_pool(name="const", bufs=1))
        W0s = cpool.tile([128, 2, NHID + 16], F16)
        nc.sync.dma_start(out=W0s[:], in_=W0i.rearrange("k p n -> p k n"))
        W1s = cpool.tile([128, 512], F16)
        nc.sync.dma_start(out=W1s[:], in_=W1i[:])
        W1as = cpool.tile([128, 16], F16)
        nc.sync.dma_start(out=W1as[:], in_=W1ai[:])
        b0s = cpool.tile([128, NHID], F32)
        nc.sync.dma_start(out=b0s[:], in_=b0bi[:])
        b1s = cpool.tile([128, NCLASS], F32)
        nc.sync.dma_start(out=b1s[:], in_=b1bi[:])
        idents = cpool.tile([128, 128], F16)
        nc.sync.dma_start(out=idents[:], in_=identi[:])
        colios = cpool.tile([128, 128], F16)
        nc.sync.dma_start(out=colios[:], in_=colioi[:])

        pid = nc.partition_id(engines=[mybir.EngineType.SP])
        sregs = [nc.sync.snap(((j + pid) % NCORES) * 128)
                 for j in range(NCORES)]

        # ---------------- phase A: layer-0 tables (replicated) ------------
        with ExitStack() as pa:
            xp = pa.enter_context(tc.tile_pool(name="pa_x", bufs=4))
            pp = pa.enter_context(
                tc.tile_pool(name="pa_ps", bufs=2, space="PSUM"))
            rp = pa.enter_context(tc.tile_pool(name="pa_row", bufs=4))
            assert gt % 4 == 0
            for gq in range(gt // 4):
                # two 2-group units per load to halve DMA issue count
                xa = xp.tile([128, 2, 2, 2, 128], F16, tag="xa")
                leng = nc.sync if gq % 2 else nc.gpsimd
                leng.dma_start(
                    out=xa[:],
                    in_=xT[2 * gq:2 * gq + 2]
                    .rearrange("G p g k f -> p G g k f"))
                for G in range(2):
                    gg = 2 * gq + G
                    row = rp.tile([128, 2, T0W], F16, tag="row")
                    for g2 in range(2):
                        ps = pp.tile([128, HID16], F32, tag=f"ps{g2}")
                        for k in range(2):
                            nc.tensor.matmul(ps[:], xa[:, G, g2, k, :],
                                             W0s[:, k, :],
                                             start=(k == 0), stop=(k == 1))
                        eng2 = nc.vector.tensor_copy if g2 else nc.scalar.copy
                        eng2(row[:, g2, 0:HID16], ps[:])
                    eng = nc.gpsimd if gg % 2 else nc.sync
                    g0 = 2 * gg * 128
                    if g0 + 256 <= SPLIT:
                        eng.dma_start(
                            out=t0lo[g0:g0 + 256, 0:HID16]
                            .rearrange("(g p) w -> p g w", p=128),
                            in_=row[:, :, 0:HID16])
                    elif g0 >= SPLIT:
                        o = g0 - SPLIT
                        eng.dma_start(
                            out=t0hi[o:o + 256, 0:HID16]
                            .rearrange("(g p) w -> p g w", p=128),
                            in_=row[:, :, 0:HID16])
                    else:
                        # group straddles the lo/hi split inside tile g2=1
                        cut = SPLIT - g0 - 128
                        eng.dma_start(out=t0lo[g0:g0 + 128, 0:HID16],
                                      in_=row[:, 0, 0:HID16])
                        eng.dma_start(
                            out=t0lo[g0 + 128:SPLIT, 0:HID16],
                            in_=row[0:cut, 1, 0:HID16])
                        eng.dma_start(
                            out=t0hi[0:256 - 128 - cut, 0:HID16],
                            in_=row[cut:128, 1, 0:HID16])

        # ---------------- shared edge phase -------------------------------
        def edge_phase(layer, post_fn, fin, hook=None):
            if layer == 0:
                tbl_lo, tbl_hi, trow, fdim = t0lo, t0hi, T0W, NHID
                gdt, adt, awcols = F16, F16, 128
                awin = t0lo[:, 128:256]
            else:
                tbl_lo, tbl_hi, trow, fdim = t1lo, t1hi, T1W, 512
                gdt, adt, awcols = F16, F16, 128
                awin = t1lo[:, 512:640]
            D = fdim // HEADS
            with ExitStack() as pb:
                ip = pb.enter_context(
                    tc.tile_pool(name=f"ix{layer}", bufs=4))
                gp = pb.enter_context(
                    tc.tile_pool(name=f"gg{layer}", bufs=4))
                apl = pb.enter_context(
                    tc.tile_pool(name=f"ga{layer}", bufs=3))
                rp2 = pb.enter_context(
                    tc.tile_pool(name=f"rh{layer}", bufs=3))
                pp2 = pb.enter_context(
                    tc.tile_pool(name=f"ps{layer}", bufs=2, space="PSUM"))
                op = pb.enter_context(
                    tc.tile_pool(name=f"po{layer}", bufs=3))
                for t in range(lt):
                    NL, NH = NLs[t], NHs[t]
                    CH = NL + NH
                    NLI, NHI = NL * 128, NH * 128
                    o_ih = NL * 8
                    o_ea = CH * 8
                    o_dr = CH * 16
                    ep = ip.tile([128, EPW[t]], I16, tag="ep")
                    nc.sync.dma_start(
                        out=ep[:], in_=epacki[:, EOFF[t]:EOFF[t + 1]])
                    il = ep[:, 0:NL * 8]
                    ih = ep[:, o_ih:o_ih + NH * 8]
                    ea = ep[:, o_ea:o_ea + CH * 8]
                    dr8 = (ep[:, o_dr:o_dr + CH * 8].bitcast(F16)
                           .rearrange("p (c e) -> p c e", e=8))

                    G = gp.tile([128, CH, trow], gdt, tag="G")
                    nc.gpsimd.dma_gather(G[:, 0:NL, :], tbl_lo[:], il,
                                         NLI, reg_of(NLI), trow,
                                         elem_step=trow)
                    nc.gpsimd.dma_gather(G[:, NL:CH, :], tbl_hi[:],
                                         ih, NHI, reg_of(NHI), trow,
                                         elem_step=trow)
                    # dst-alpha gathers: 256B column window of the lo table
                    # (<=1024 idx per SWDGE call)
                    A = apl.tile([128, CH, awcols], adt, tag="A")
                    nc.gpsimd.dma_gather(A[:, 0:NL, :], awin,
                                         ea[:, 0:NL * 8], NLI, reg_of(NLI),
                                         awcols, elem_step=trow)
                    nc.gpsimd.dma_gather(A[:, NL:CH, :], awin,
                                         ea[:, NL * 8:CH * 8], NHI,
                                         reg_of(NHI), awcols, elem_step=trow)
                    g_as = G[:, :, fdim:fdim + 8]
                    a_ad = A[:, :, 8:16]
                    g_f = G[:, :, 0:fdim]

                    inc = rp2.tile([128, CH, 128], F16, tag="inc")
                    nc.vector.tensor_tensor(
                        out=inc[:].rearrange("p c (g e) -> p c g e", e=8),
                        in0=dr8.unsqueeze(2)
                        .broadcast_to([128, CH, 16, 8]),
                        in1=colios[:].rearrange("p (g e) -> p g e", e=8)
                        .unsqueeze(1).broadcast_to([128, CH, 16, 8]),
                        op=mybir.AluOpType.is_equal)
                    EX = rp2.tile([128, CH, 8], F16, tag="EX")
                    nc.vector.tensor_tensor(
                        out=EX[:], in0=g_as, in1=a_ad,
                        op=mybir.AluOpType.add)
                    if sim_safe:
                        EXr = rp2.tile([128, CH, 8], F16, tag="EXr")
                        nc.scalar.activation(
                            EXr[:], EX[:],
                            mybir.ActivationFunctionType.Relu, scale=0.8)
                        nc.vector.tensor_scalar_mul(EX[:], EX[:], SLOPE)
                        nc.vector.tensor_tensor(
                            out=EX[:], in0=EX[:], in1=EXr[:],
                            op=mybir.AluOpType.add)
                    else:
                        nc.scalar.activation(
                            EX[:], EX[:],
                            mybir.ActivationFunctionType.Prelu, alpha=SLOPE)
                    nc.scalar.activation(
                        EX[:], EX[:], mybir.ActivationFunctionType.Exp)

                    R = rp2.tile([128, CH, fdim], F16, tag="R")
                    nc.vector.tensor_tensor(
                        out=R[:].rearrange("p c (d h) -> p c d h", h=HEADS),
                        in0=g_f.rearrange("p c (d h) -> p c d h", h=HEADS),
                        in1=EX[:].unsqueeze(2)
                        .broadcast_to([128, CH, D, HEADS]),
                        op=mybir.AluOpType.mult)

                    P1 = pp2.tile([128, fdim], F32, tag="P1")
                    P2 = pp2.tile([128, 8], F32, tag="P2")
                    for ch in range(CH):
                        nc.tensor.matmul(P1[:], inc[:, ch, :],
                                         R[:, ch, 0:fdim],
                                         start=(ch == 0),
                                         stop=(ch == CH - 1))
                    for ch in range(CH):
                        nc.tensor.matmul(P2[:], inc[:, ch, :],
                                         EX[:, ch, :],
                                         start=(ch == 0),
                                         stop=(ch == CH - 1))
                    post_fn(t, P1, P2, op, pp2, fin)
                    if hook is not None:
                        hook(t)

        # ---- L0 post: softmax-div, +b0, ELU, transpose, store ------------
        def post0(t, P1, P2, op, pp2, fin):
            rows = 128 if t < lt - 1 else LAST_ROWS
            r8 = op.tile([128, 8], F32, tag="r8")
            nc.vector.tensor_scalar_add(r8[:], P2[:], 1e-16)
            nc.vector.reciprocal(r8[:], r8[:])
            z = op.tile([128, NHID], F32, tag="z")
            nc.vector.tensor_tensor(
                out=z[:].rearrange("p (d h) -> p d h", h=HEADS),
                in0=P1[:].rearrange("p (d h) -> p d h", h=HEADS),
                in1=r8[:].unsqueeze(1).broadcast_to([128, 16, HEADS]),
                op=mybir.AluOpType.mult)
            nc.vector.tensor_tensor(out=z[:], in0=z[:], in1=b0s[:],
                                    op=mybir.AluOpType.add)
            zm = op.tile([128, NHID], F32, tag="zm")
            nc.vector.tensor_scalar_min(zm[:], z[:], 0.0)
            nc.scalar.activation(zm[:], zm[:],
                                 mybir.ActivationFunctionType.Exp)
            zp = op.tile([128, NHID], F32, tag="zp")
            nc.vector.tensor_scalar_max(zp[:], z[:], 0.0)
            nc.vector.tensor_tensor(out=zp[:], in0=zp[:], in1=zm[:],
                                    op=mybir.AluOpType.add)
            h1 = op.tile([128, NHID], F16, tag="h1")
            nc.vector.tensor_scalar_add(h1[:], zp[:], -1.0)
            pst = pp2.tile([128, 128], F16, tag="pst")
            nc.tensor.transpose(pst[:], h1[:], idents[:])
            hT = op.tile([128, 128], F8, tag="hT")
            nc.vector.tensor_copy(hT[:], pst[:])
            k = min(t // 12, NCHUNK - 1)
            col = (t - k * 12) * 128
            nc.sync.dma_start(
                out=aginc[k][:, col:col + rows], in_=hT[:, 0:rows])

        # chunked AllGather: issued from inside the B loop as soon as a
        # chunk's 12 tiles land, overlapping the collective with B and D
        coll_at = {15: 0, 27: 1, 39: 2, 48: 3}

        def issue_coll(k):
            nc.gpsimd.collective_compute(
                "AllGather", mybir.AluOpType.bypass,
                replica_groups=[list(range(NCORES))],
                ins=[aginc[k][:]], outs=[agoutc[k][:]])

        def b_hook(t):
            if t in coll_at:
                issue_coll(coll_at[t])

        if "B" in phases:
            edge_phase(0, post0, None, hook=b_hook)
            issue_coll(4)

        # ---------------- phase D: layer-1 tables (chunk-major) -----------
        with ExitStack() as pd:
            xp1 = pd.enter_context(tc.tile_pool(name="pd_x", bufs=4))
            pp1 = pd.enter_context(
                tc.tile_pool(name="pd_ps", bufs=2, space="PSUM"))
            rp1 = pd.enter_context(tc.tile_pool(name="pd_row", bufs=2))
            dunits = ([(k, r) for k in range(NCHUNK) for r in range(NCORES)]
                      if "D" in phases else [])
            wengs = [nc.sync, nc.scalar, nc.gpsimd]
            for k, r in dunits:
                base = r * NLOC + k * CCOLS
                w = CW[k]
                if w == CCOLS:
                    hx8 = xp1.tile([128, CCOLS], F8, tag="hx8")
                    nc.sync.dma_start(
                        out=hx8[:],
                        in_=agoutc[k][bass.ds(sregs[r], 128), :])
                    hx = xp1.tile([128, CCOLS], F16, tag="hx")
                    nc.gpsimd.tensor_copy(hx[:], hx8[:])
                    row = rp1.tile([128, 12, 528], F16, tag="row")
                    for g2 in range(6):
                        # each q's 512-col matmul exactly fills one PSUM
                        # bank (outputs must not cross 2KB banks)
                        psf = pp1.tile([128, 2, 512], F32, tag="psf")
                        psa = pp1.tile([128, 2, 16], F32, tag="psa")
                        for q in range(2):
                            hs = hx[:, (g2 * 2 + q) * 128:
                                    (g2 * 2 + q + 1) * 128]
                            nc.tensor.matmul(psf[:, q, :], hs, W1s[:],
                                             start=True, stop=True)
                            nc.tensor.matmul(psa[:, q, :], hs,
                                             W1as[:], start=True, stop=True)
                        feng = nc.scalar.copy if g2 % 2 else \
                            nc.vector.tensor_copy
                        feng(row[:, 2 * g2:2 * g2 + 2, 0:512], psf[:])
                        nc.vector.tensor_copy(
                            row[:, 2 * g2:2 * g2 + 2, 512:528], psa[:])
                    eng = wengs[(k * NCORES + r) % 3]
                    if r < 4:
                        eng.dma_start(
                            out=t1lo[base:base + CCOLS, 0:528]
                            .rearrange("(g p) w -> p g w", p=128),
                            in_=row[:])
                    else:
                        o = base - SPLIT
                        eng.dma_start(
                            out=t1hi[o:o + CCOLS, 0:528]
                            .rearrange("(g p) w -> p g w", p=128),
                            in_=row[:])
                else:
                    hx8 = xp1.tile([128, w], F8, tag="hx8t")
                    nc.sync.dma_start(
                        out=hx8[:],
                        in_=agoutc[k][bass.ds(sregs[r], 128), :])
                    hx = xp1.tile([128, w], F16, tag="hxt")
                    nc.gpsimd.tensor_copy(hx[:], hx8[:])
                    psf = pp1.tile([128, 2, 512], F32, tag="psf")
                    psa = pp1.tile([128, 2, 16], F32, tag="psa")
                    nc.tensor.matmul(psf[0:w, 0, :], hx[:], W1s[:],
                                     start=True, stop=True)
                    nc.tensor.matmul(psa[0:w, 0, :], hx[:], W1as[:],
                                     start=True, stop=True)
                    row = rp1.tile([128, 12, 528], F16, tag="row")
                    nc.scalar.copy(row[0:w, 0, 0:512], psf[0:w, 0, :])
                    nc.vector.tensor_copy(row[0:w, 0, 512:528],
                                          psa[0:w, 0, :])
                    if r < 4:
                        nc.sync.dma_start(out=t1lo[base:base + w, 0:528],
                                          in_=row[0:w, 0, :])
                    else:
                        o = base - SPLIT
                        nc.sync.dma_start(out=t1hi[o:o + w, 0:528],
                                          in_=row[0:w, 0, :])

        # ---------------- phase E: layer-1 edges + epilogue ---------------
        def post1(t, P1, P2, op, pp2, fin):
            zbig, nmxb, seb = fin
            r8 = op.tile([128, 8], F32, tag="r8")
            nc.vector.tensor_scalar_add(r8[:], P2[:], 1e-16)
            nc.vector.reciprocal(r8[:], r8[:])
            nc.vector.tensor_scalar_mul(r8[:], r8[:], 1.0 / HEADS)
            zw = op.tile([128, 512], F32, tag="zw")
            nc.vector.tensor_tensor(
                out=zw[:].rearrange("p (d h) -> p d h", h=HEADS),
                in0=P1[:].rearrange("p (d h) -> p d h", h=HEADS),
                in1=r8[:].unsqueeze(1).broadcast_to([128, 64, HEADS]),
                op=mybir.AluOpType.mult)
            z = zbig[:, t * NCLASS:(t + 1) * NCLASS]
            nc.vector.reduce_sum(
                z, zw[:].rearrange("p (d h) -> p d h", h=HEADS),
                axis=mybir.AxisListType.X)
            nc.vector.tensor_tensor(out=z, in0=z, in1=b1s[:],
                                    op=mybir.AluOpType.add)
            nmx = nmxb[:, t:t + 1]
            nc.vector.reduce_max(nmx, z, axis=mybir.AxisListType.X,
                                 negate=True)
            ez = op.tile([128, NCLASS], F32, tag="ez")
            nc.scalar.activation(ez[:], z,
                                 mybir.ActivationFunctionType.Exp,
                                 bias=nmx, accum_out=seb[:, t:t + 1])

        if "E" in phases:
            fpool = stk.enter_context(tc.tile_pool(name="fin", bufs=1))
            zbig = fpool.tile([128, lt * NCLASS], F32)
            nmxb = fpool.tile([128, lt], F32)
            seb = fpool.tile([128, lt], F32)
            edge_phase(1, post1, (zbig, nmxb, seb))
            # batched log-softmax tail: one Ln + two broadcast ops + 2 DMAs
            nc.scalar.activation(seb[:], seb[:],
                                 mybir.ActivationFunctionType.Ln)
            nc.vector.tensor_tensor(
                out=zbig[:].rearrange("p (t c) -> p t c", c=NCLASS),
                in0=zbig[:].rearrange("p (t c) -> p t c", c=NCLASS),
                in1=nmxb[:].unsqueeze(-1).broadcast_to([128, lt, NCLASS]),
                op=mybir.AluOpType.add)
            nc.vector.tensor_tensor(
                out=zbig[:].rearrange("p (t c) -> p t c", c=NCLASS),
                in0=zbig[:].rearrange("p (t c) -> p t c", c=NCLASS),
                in1=seb[:].unsqueeze(-1).broadcast_to([128, lt, NCLASS]),
                op=mybir.AluOpType.subtract)
            nfull = (lt - 1) * 128
            rlast = LAST_ROWS if lt == LT else 128
            nc.sync.dma_start(
                out=out[0:nfull, :].rearrange("(t p) c -> p t c", p=128),
                in_=zbig[:].rearrange("p (t c) -> p t c", c=NCLASS)
                [:, 0:lt - 1, :])
            nc.sync.dma_start(
                out=out[nfull:nfull + rlast, :],
                in_=zbig[0:rlast, (lt - 1) * NCLASS:lt * NCLASS])

    nc.compile()
    return nc


# --------------------------------------------------------------------------
# entry point
# --------------------------------------------------------------------------

def kernel(**inputs) -> np.ndarray:
    NLs, NHs, in_maps = _prep_inputs(**inputs)
    key = (tuple(NLs), tuple(NHs))
    if key not in _cache:
        _cache[key] = build(NLs, NHs)
    nc = _cache[key]
    res = run_bass_kernel_spmd(nc, in_maps, list(range(NCORES)))
    return np.concatenate([res.results[c]["out"] for c in range(NCORES)], 0)


# revision 9
# speedup vs baseline: 1.1570x; 1.0363x over previous
"""2-layer GAT (nn_GAT_31490700214331) on 8 Trainium2 NeuronCores.

Strategy (dst-sharded, SPMD, per-core-rotated node layout):
  - Nodes are block-partitioned: core c owns nodes [c*6250, (c+1)*6250).
  - Every table on core c uses a ROTATED row order: node n lives at row
    (n - c*6250) mod 50000, so each core's own nodes are rows 0..6249 and
    the single SPMD program has no core-dependent offsets.
  - Layer-0 features (h0 = x @ W0) + attention alphas are computed
    replicated on every core into a rotated f16 DRAM table; edges are
    grouped by dst tile (128 dsts), per-tile chunk counts specialized to
    the actual edge counts (max over cores), and source rows fetched with
    dma_gather through lo/hi table views (int16 indices < 32768).
  - Per-edge dst alphas come from one merged dma_gather over a 256B
    column window of the lo table.
  - Edge softmax (safe without segment-max: |e| <= ~5) and the weighted
    aggregation fuse into per-chunk 128x128 incidence matmuls in PSUM.
  - The ELU'd hidden state is AllGather'd in fp8(e3m4) chunks overlapped
    with phase B, rotated into per-core order, and layer 1 runs on an
    fp8 feature table (f16 alphas riding in the same 768B row) gathered
    at 768B/edge.
  - alpha projections fold into the weight matmuls on the host:
    h @ blockdiag(a) == x @ (W @ blockdiag(a)).

Self-contained: call kernel(**inputs) with the full-problem arrays.
"""
import numpy as np
from contextlib import ExitStack

import concourse.bacc as bacc
import concourse.bass as bass
import concourse.mybir as mybir
from concourse.tile import TileContext
from concourse.bass_utils import run_bass_kernel_spmd

F16 = mybir.dt.float16
F32 = mybir.dt.float32
F8 = mybir.dt.float8e3          # e3m4: 4 mantissa bits, max 15.5
I16 = mybir.dt.int16

N = 50000
NFEAT = 256
NHID = 128
NCLASS = 64
HEADS = 8
SLOPE = 0.2
NCORES = 8
NLOC = N // NCORES           # 6250
LT = (NLOC + 127) // 128     # 49 local dst tiles
LAST_ROWS = NLOC - (LT - 1) * 128   # 106 rows in the last tile
GT = 392                     # global node tiles (392*128 = 50176)
GROWS = GT * 128
SPLIT = 25000                # low/high gather-table split (4 core blocks)
CCOLS = 1536                 # collective chunk width (12 B-tiles)
NCHUNK = 5                   # 4 full chunks + 106-col tail
SENT = 300.0                 # dst_rel sentinel for padding slots
T0W = 256                    # t0 row: [h0(128)|as0(8)|ad0(8)|junk] f16
T1W = 640                    # t1 row: [h1(512)|as1(8)|ad1(8)|junk] f16

_cache = {}


# --------------------------------------------------------------------------
# host-side preparation
# --------------------------------------------------------------------------

def _wrap_idx(idx):
    """[n] int -> [128, n//16] int16 wrapped gather-index layout."""
    n = idx.shape[0]
    assert n % 16 == 0
    w = idx.reshape(n // 16, 16).T.astype(np.int16)
    return np.tile(w, (8, 1))


def _prep_edges(src, dst):
    cores = []
    for c in range(NCORES):
        m = (dst >= c * NLOC) & (dst < (c + 1) * NLOC)
        s = src[m].astype(np.int64)
        d = dst[m].astype(np.int64) - c * NLOC
        order = np.argsort(d, kind="stable")
        s, d = s[order], d[order]
        s_rot = (s - c * NLOC) % N
        tiles = []
        for t in range(LT):
            sel = (d >= t * 128) & (d < (t + 1) * 128)
            st, dt = s_rot[sel], d[sel] - t * 128
            lo = st < SPLIT
            tiles.append((st[lo], dt[lo], st[~lo] - SPLIT, dt[~lo]))
        cores.append(tiles)
    # per-tile chunk counts (max over cores so the SPMD program is shared)
    NLs, NHs = [], []
    for t in range(LT):
        nl = max(len(cores[c][t][0]) for c in range(NCORES))
        nh = max(len(cores[c][t][2]) for c in range(NCORES))
        NLs.append(max(1, (nl + 127) // 128))
        NHs.append(max(1, (nh + 127) // 128))
        assert NLs[t] * 128 <= 1024 and NHs[t] * 128 <= 1024

    out = []
    for c in range(NCORES):
        blocks = []
        for t in range(LT):
            NL, NH = NLs[t], NHs[t]
            CH = NL + NH
            sl, dl, sh, dh = cores[c][t]
            il = np.zeros(NL * 128, np.int64)
            il[: len(sl)] = sl
            ih = np.zeros(NH * 128, np.int64)
            ih[: len(sh)] = sh
            aa = np.zeros(CH * 128, np.int64)
            aa[: len(dl)] = t * 128 + dl
            aa[NL * 128: NL * 128 + len(dh)] = t * 128 + dh
            rl = np.full(NL * 128, SENT)
            rl[: len(dl)] = dl
            rh = np.full(NH * 128, SENT)
            rh[: len(dh)] = dh
            r = np.concatenate([rl, rh]).reshape(CH, 128).T
            drel = np.broadcast_to(
                r.astype(np.float16)[:, :, None],
                (128, CH, 8)).reshape(128, CH * 8)
            blocks.append(np.concatenate(
                [_wrap_idx(il), _wrap_idx(ih), _wrap_idx(aa),
                 np.ascontiguousarray(drel).view(np.int16)], axis=1))
        out.append(dict(epack=np.ascontiguousarray(
            np.concatenate(blocks, axis=1))))
    return NLs, NHs, out


def _prep_inputs(x, edge_index, W0, a_src0, a_dst0, b0, W1, a_src1, a_dst1,
                 b1):
    src = np.asarray(edge_index[0]).astype(np.int64)
    dst = np.asarray(edge_index[1]).astype(np.int64)
    NLs, NHs, edata = _prep_edges(src, dst)

    def bd(a):  # [H, D] -> blockdiag [H*D, H]
        a = np.asarray(a, np.float32)
        H, D = a.shape
        m = np.zeros((H * D, H), np.float32)
        for h in range(H):
            m[h * D:(h + 1) * D, h] = a[h]
        return m

    W0 = np.asarray(W0, np.float32)
    W1 = np.asarray(W1, np.float32)
    W0a = np.concatenate([W0 @ bd(a_src0), W0 @ bd(a_dst0)], 1)  # [256, 16]
    # head-innermost feature interleave: new col d*8+h <- old col h*D+d
    perm0 = np.array([(f % 8) * 16 + f // 8 for f in range(128)])
    perm1 = np.array([(f % 8) * 64 + f // 8 for f in range(512)])
    W0cat = np.concatenate([W0[:, perm0], W0a], 1)               # [256, 144]
    W1a = np.concatenate([W1 @ bd(a_src1), W1 @ bd(a_dst1)], 1)  # [128, 16]

    x = np.asarray(x, np.float32)
    ident = np.eye(128, dtype=np.float16)
    colio = np.tile(np.arange(128, dtype=np.float16)[None, :], (128, 1))
    b0b = np.tile(np.asarray(b0, np.float32)[None, :], (128, 1))
    b1b = np.tile(np.asarray(b1, np.float32)[None, :], (128, 1))

    in_maps = []
    for c in range(NCORES):
        rot = np.roll(np.arange(N), -c * NLOC)
        xr = np.zeros((GROWS, NFEAT), np.float16)
        xr[:N] = x[rot].astype(np.float16)
        # [gg, 128(j feat), 2(g), 2(k), 128(p node)]: partition = feature,
        # per-partition contiguous 1KB runs
        xtt = (xr.reshape(GROWS // 256, 2, 128, 2, 128)
               .transpose(0, 4, 1, 3, 2))
        m = dict(
            xT=np.ascontiguousarray(xtt),
            W0=np.ascontiguousarray(
                W0cat.astype(np.float16).reshape(2, 128, NHID + 16)),
            W1=np.ascontiguousarray(W1[perm0][:, perm1].astype(np.float16)),
            W1a=np.ascontiguousarray(W1a[perm0].astype(np.float16)),
            b0b=np.ascontiguousarray(b0b[:, perm0]), b1b=b1b,
            ident=ident, colio=colio,
            **edata[c],
        )
        in_maps.append(m)
    return NLs, NHs, in_maps


# --------------------------------------------------------------------------
# device program
# --------------------------------------------------------------------------

def build(NLs, NHs, lt=LT, gt=GT, debug=False, phases="ABCDE",
          sim_safe=False):
    NLs, NHs = list(NLs), list(NHs)
    HID16 = NHID + 16
    EPW = [(NLs[t] + NHs[t]) * 24 for t in range(lt)]
    EOFF = np.concatenate([[0], np.cumsum(EPW)]).astype(int)
    nc = bacc.Bacc("TRN2")
    xT = nc.dram_tensor("xT", [GROWS // 256, 128, 2, 2, 128], F16,
                        kind="ExternalInput")
    W0i = nc.dram_tensor("W0", [2, 128, NHID + 16], F16,
                         kind="ExternalInput")
    W1i = nc.dram_tensor("W1", [NHID, 512], F16, kind="ExternalInput")
    W1ai = nc.dram_tensor("W1a", [NHID, 16], F16, kind="ExternalInput")
    b0bi = nc.dram_tensor("b0b", [128, NHID], F32, kind="ExternalInput")
    b1bi = nc.dram_tensor("b1b", [128, NCLASS], F32, kind="ExternalInput")
    identi = nc.dram_tensor("ident", [128, 128], F16, kind="ExternalInput")
    colioi = nc.dram_tensor("colio", [128, 128], F16, kind="ExternalInput")
    epacki = nc.dram_tensor("epack", [128, int(EOFF[-1])], I16,
                            kind="ExternalInput")
    out = nc.dram_tensor("out", [NLOC, NCLASS], F32, kind="ExternalOutput")

    with TileContext(nc) as tc, ExitStack() as stk:
        regs = {}

        def reg_of(n):
            if n not in regs:
                regs[n] = nc.gpsimd.to_reg(n)
            return regs[n]

        dpool = stk.enter_context(
            tc.tile_pool(name="dram", bufs=1, space="DRAM"))
        t0lo = dpool.tile([SPLIT, T0W], F16, tag="t0lo")
        t0hi = dpool.tile([GROWS - SPLIT, T0W], F16, tag="t0hi")
        t1lo = dpool.tile([SPLIT, T1W], F16, tag="t1lo")
        t1hi = dpool.tile([GROWS - SPLIT, T1W], F16, tag="t1hi")
        CW = [CCOLS] * 4 + [NLOC - 4 * CCOLS]   # 4x1536 + 106
        aginc = [dpool.tile([128, CW[k]], F8, tag=f"agin{k}",
                            name=f"agin{k}")
                 for k in range(NCHUNK)]
        agoutc = [dpool.tile([NCORES * 128, CW[k]], F8, tag=f"agout{k}",
                             addr_space="Shared", name=f"agout{k}")
                  for k in range(NCHUNK)]

        cpool = stk.enter_context(tc.tile_pool(name="const", bufs=1))
        W0s = cpool.tile([128, 2, NHID + 16], F16)
        nc.sync.dma_start(out=W0s[:], in_=W0i.rearrange("k p n -> p k n"))
        W1s = cpool.tile([128, 512], F16)
        nc.sync.dma_start(out=W1s[:], in_=W1i[:])
        W1as = cpool.tile([128, 16], F16)
        nc.sync.dma_start(out=W1as[:], in_=W1ai[:])
        b0s = cpool.tile([128, NHID], F32)
        nc.sync.dma_start(out=b0s[:], in_=b0bi[:])
        b1s = cpool.tile([128, NCLASS], F32)
        nc.sync.dma_start(out=b1s[:], in_=b1bi[:])
        idents = cpool.tile([128, 128], F16)
        nc.sync.dma_start(out=idents[:], in_=identi[:])
        colios = cpool.tile([128, 128], F16)
        nc.sync.dma_start(out=colios[:], in_=colioi[:])

        pid = nc.partition_id(engines=[mybir.EngineType.SP])
        sregs = [nc.sync.snap(((j + pid) % NCORES) * 128)
                 for j in range(NCORES)]

        # ---------------- phase A: layer-0 tables (replicated) ------------
        with ExitStack() as pa:
            xp = pa.enter_context(tc.tile_pool(name="pa_x", bufs=4))
            pp = pa.enter_context(
                tc.tile_pool(name="pa_ps", bufs=2, space="PSUM"))
            rp = pa.enter_context(tc.tile_pool(name="pa_row", bufs=4))
            assert gt % 4 == 0
            for gq in range(gt // 4):
                # two 2-group units per load to halve DMA issue count
                xa = xp.tile([128, 2, 2, 2, 128], F16, tag="xa")
                leng = nc.sync if gq % 2 else nc.gpsimd
                leng.dma_start(
                    out=xa[:],
                    in_=xT[2 * gq:2 * gq + 2]
                    .rearrange("G p g k f -> p G g k f"))
                for G in range(2):
                    gg = 2 * gq + G
                    row = rp.tile([128, 2, T0W], F16, tag="row")
                    for g2 in range(2):
                        ps = pp.tile([128, HID16], F32, tag=f"ps{g2}")
                        for k in range(2):
                            nc.tensor.matmul(ps[:], xa[:, G, g2, k, :],
                                             W0s[:, k, :],
                                             start=(k == 0), stop=(k == 1))
                        eng2 = nc.vector.tensor_copy if g2 else nc.scalar.copy
                        eng2(row[:, g2, 0:HID16], ps[:])
                    eng = nc.gpsimd if gg % 2 else nc.sync
                    g0 = 2 * gg * 128
                    if g0 + 256 <= SPLIT:
                        eng.dma_start(
                            out=t0lo[g0:g0 + 256, 0:HID16]
                            .rearrange("(g p) w -> p g w", p=128),
                            in_=row[:, :, 0:HID16])
                    elif g0 >= SPLIT:
                        o = g0 - SPLIT
                        eng.dma_start(
                            out=t0hi[o:o + 256, 0:HID16]
                            .rearrange("(g p) w -> p g w", p=128),
                            in_=row[:, :, 0:HID16])
                    else:
                        # group straddles the lo/hi split inside tile g2=1
                        cut = SPLIT - g0 - 128
                        eng.dma_start(out=t0lo[g0:g0 + 128, 0:HID16],
                                      in_=row[:, 0, 0:HID16])
                        eng.dma_start(
                            out=t0lo[g0 + 128:SPLIT, 0:HID16],
                            in_=row[0:cut, 1, 0:HID16])
                        eng.dma_start(
                            out=t0hi[0:256 - 128 - cut, 0:HID16],
                            in_=row[cut:128, 1, 0:HID16])

        # ---------------- shared edge phase -------------------------------
        def edge_phase(layer, post_fn, fin, hook=None):
            if layer == 0:
                tbl_lo, tbl_hi, trow, fdim = t0lo, t0hi, T0W, NHID
                gdt, adt, awcols = F16, F16, 128
                awin = t0lo[:, 128:256]
            else:
                tbl_lo, tbl_hi, trow, fdim = t1lo, t1hi, T1W, 512
                gdt, adt, awcols = F16, F16, 128
                awin = t1lo[:, 512:640]
            D = fdim // HEADS
            with ExitStack() as pb:
                ip = pb.enter_context(
                    tc.tile_pool(name=f"ix{layer}", bufs=4))
                gp = pb.enter_context(
                    tc.tile_pool(name=f"gg{layer}", bufs=4))
                apl = pb.enter_context(
                    tc.tile_pool(name=f"ga{layer}", bufs=3))
                rp2 = pb.enter_context(
                    tc.tile_pool(name=f"rh{layer}", bufs=3))
                pp2 = pb.enter_context(
                    tc.tile_pool(name=f"ps{layer}", bufs=2, space="PSUM"))
                op = pb.enter_context(
                    tc.tile_pool(name=f"po{layer}", bufs=3))
                for t in range(lt):
                    NL, NH = NLs[t], NHs[t]
                    CH = NL + NH
                    NLI, NHI = NL * 128, NH * 128
                    o_ih = NL * 8
                    o_ea = CH * 8
                    o_dr = CH * 16
                    ep = ip.tile([128, EPW[t]], I16, tag="ep")
                    nc.sync.dma_start(
                        out=ep[:], in_=epacki[:, EOFF[t]:EOFF[t + 1]])
                    il = ep[:, 0:NL * 8]
                    ih = ep[:, o_ih:o_ih + NH * 8]
                    ea = ep[:, o_ea:o_ea + CH * 8]
                    dr8 = (ep[:, o_dr:o_dr + CH * 8].bitcast(F16)
                           .rearrange("p (c e) -> p c e", e=8))

                    G = gp.tile([128, CH, trow], gdt, tag="G")
                    nc.gpsimd.dma_gather(G[:, 0:NL, :], tbl_lo[:], il,
                                         NLI, reg_of(NLI), trow,
                                         elem_step=trow)
                    nc.gpsimd.dma_gather(G[:, NL:CH, :], tbl_hi[:],
                                         ih, NHI, reg_of(NHI), trow,
                                         elem_step=trow)
                    # dst-alpha gathers: 256B column window of the lo table
                    # (<=1024 idx per SWDGE call)
                    A = apl.tile([128, CH, awcols], adt, tag="A")
                    nc.gpsimd.dma_gather(A[:, 0:NL, :], awin,
                                         ea[:, 0:NL * 8], NLI, reg_of(NLI),
                                         awcols, elem_step=trow)
                    nc.gpsimd.dma_gather(A[:, NL:CH, :], awin,
                                         ea[:, NL * 8:CH * 8], NHI,
                                         reg_of(NHI), awcols, elem_step=trow)
                    g_as = G[:, :, fdim:fdim + 8]
                    a_ad = A[:, :, 8:16]
                    g_f = G[:, :, 0:fdim]

                    inc = rp2.tile([128, CH, 128], F16, tag="inc")
                    nc.vector.tensor_tensor(
                        out=inc[:].rearrange("p c (g e) -> p c g e", e=8),
                        in0=dr8.unsqueeze(2)
                        .broadcast_to([128, CH, 16, 8]),
                        in1=colios[:].rearrange("p (g e) -> p g e", e=8)
                        .unsqueeze(1).broadcast_to([128, CH, 16, 8]),
                        op=mybir.AluOpType.is_equal)
                    EX = rp2.tile([128, CH, 8], F16, tag="EX")
                    nc.vector.tensor_tensor(
                        out=EX[:], in0=g_as, in1=a_ad,
                        op=mybir.AluOpType.add)
                    if sim_safe:
                        EXr = rp2.tile([128, CH, 8], F16, tag="EXr")
                        nc.scalar.activation(
                            EXr[:], EX[:],
                            mybir.ActivationFunctionType.Relu, scale=0.8)
                        nc.vector.tensor_scalar_mul(EX[:], EX[:], SLOPE)
                        nc.vector.tensor_tensor(
                            out=EX[:], in0=EX[:], in1=EXr[:],
                            op=mybir.AluOpType.add)
                    else:
                        nc.scalar.activation(
                            EX[:], EX[:],
                            mybir.ActivationFunctionType.Prelu, alpha=SLOPE)
                    nc.scalar.activation(
                        EX[:], EX[:], mybir.ActivationFunctionType.Exp)

                    R = rp2.tile([128, CH, fdim], F16, tag="R")
                    nc.vector.tensor_tensor(
                        out=R[:].rearrange("p c (d h) -> p c d h", h=HEADS),
                        in0=g_f.rearrange("p c (d h) -> p c d h", h=HEADS),
                        in1=EX[:].unsqueeze(2)
                        .broadcast_to([128, CH, D, HEADS]),
                        op=mybir.AluOpType.mult)

                    P1 = pp2.tile([128, fdim], F32, tag="P1")
                    P2 = pp2.tile([128, 8], F32, tag="P2")
                    for ch in range(CH):
                        nc.tensor.matmul(P1[:], inc[:, ch, :],
                                         R[:, ch, 0:fdim],
                                         start=(ch == 0),
                                         stop=(ch == CH - 1))
                    for ch in range(CH):
                        nc.tensor.matmul(P2[:], inc[:, ch, :],
                                         EX[:, ch, :],
                                         start=(ch == 0),
                                         stop=(ch == CH - 1))
                    post_fn(t, P1, P2, op, pp2, fin)
                    if hook is not None:
                        hook(t)

        # ---- L0 post: softmax-div, +b0, ELU, transpose, store ------------
        def post0(t, P1, P2, op, pp2, fin):
            rows = 128 if t < lt - 1 else LAST_ROWS
            r8 = op.tile([128, 8], F32, tag="r8")
            nc.vector.tensor_scalar_add(r8[:], P2[:], 1e-16)
            nc.vector.reciprocal(r8[:], r8[:])
            z = op.tile([128, NHID], F32, tag="z")
            nc.vector.tensor_tensor(
                out=z[:].rearrange("p (d h) -> p d h", h=HEADS),
                in0=P1[:].rearrange("p (d h) -> p d h", h=HEADS),
                in1=r8[:].unsqueeze(1).broadcast_to([128, 16, HEADS]),
                op=mybir.AluOpType.mult)
            nc.vector.tensor_tensor(out=z[:], in0=z[:], in1=b0s[:],
                                    op=mybir.AluOpType.add)
            zm = op.tile([128, NHID], F32, tag="zm")
            nc.vector.tensor_scalar_min(zm[:], z[:], 0.0)
            nc.scalar.activation(zm[:], zm[:],
                                 mybir.ActivationFunctionType.Exp)
            zp = op.tile([128, NHID], F32, tag="zp")
            nc.vector.tensor_scalar_max(zp[:], z[:], 0.0)
            nc.vector.tensor_tensor(out=zp[:], in0=zp[:], in1=zm[:],
                                    op=mybir.AluOpType.add)
            h1 = op.tile([128, NHID], F16, tag="h1")
            nc.vector.tensor_scalar_add(h1[:], zp[:], -1.0)
            pst = pp2.tile([128, 128], F16, tag="pst")
            nc.tensor.transpose(pst[:], h1[:], idents[:])
            hT = op.tile([128, 128], F8, tag="hT")
            nc.vector.tensor_copy(hT[:], pst[:])
            k = min(t // 12, NCHUNK - 1)
            col = (t - k * 12) * 128
            nc.sync.dma_start(
                out=aginc[k][:, col:col + rows], in_=hT[:, 0:rows])

        # chunked AllGather: issued from inside the B loop as soon as a
        # chunk's 12 tiles land, overlapping the collective with B and D
        def issue_coll(k):
            nc.gpsimd.collective_compute(
                "AllGather", mybir.AluOpType.bypass,
                replica_groups=[list(range(NCORES))],
                ins=[aginc[k][:]], outs=[agoutc[k][:]])

        if "B" in phases:
            edge_phase(0, post0, None)
            for k in range(NCHUNK):
                issue_coll(k)

        # ---------------- phase D: layer-1 tables (chunk-major) -----------
        with ExitStack() as pd:
            xp1 = pd.enter_context(tc.tile_pool(name="pd_x", bufs=4))
            pp1 = pd.enter_context(
                tc.tile_pool(name="pd_ps", bufs=2, space="PSUM"))
            rp1 = pd.enter_context(tc.tile_pool(name="pd_row", bufs=2))
            dunits = ([(k, r) for k in range(NCHUNK) for r in range(NCORES)]
                      if "D" in phases else [])
            wengs = [nc.sync, nc.gpsimd]
            for k, r in dunits:
                base = r * NLOC + k * CCOLS
                w = CW[k]
                if w == CCOLS:
                    hx8 = xp1.tile([128, CCOLS], F8, tag="hx8")
                    nc.sync.dma_start(
                        out=hx8[:],
                        in_=agoutc[k][bass.ds(sregs[r], 128), :])
                    hx = xp1.tile([128, CCOLS], F16, tag="hx")
                    nc.gpsimd.tensor_copy(hx[:], hx8[:])
                    row = rp1.tile([128, 12, 528], F16, tag="row")
                    for g2 in range(6):
                        # each q's 512-col matmul exactly fills one PSUM
                        # bank (outputs must not cross 2KB banks)
                        psf = pp1.tile([128, 2, 512], F32, tag="psf")
                        psa = pp1.tile([128, 2, 16], F32, tag="psa")
                        for q in range(2):
                            hs = hx[:, (g2 * 2 + q) * 128:
                                    (g2 * 2 + q + 1) * 128]
                            nc.tensor.matmul(psf[:, q, :], hs, W1s[:],
                                             start=True, stop=True)
                            nc.tensor.matmul(psa[:, q, :], hs,
                                             W1as[:], start=True, stop=True)
                        feng = nc.scalar.copy if g2 % 2 else \
                            nc.vector.tensor_copy
                        aeng = nc.vector.tensor_copy if g2 % 2 else \
                            nc.scalar.copy
                        feng(row[:, 2 * g2:2 * g2 + 2, 0:512], psf[:])
                        aeng(row[:, 2 * g2:2 * g2 + 2, 512:528], psa[:])
                    eng = wengs[(k * NCORES + r) % 2]
                    if r < 4:
                        eng.dma_start(
                            out=t1lo[base:base + CCOLS, 0:528]
                            .rearrange("(g p) w -> p g w", p=128),
                            in_=row[:])
                    else:
                        o = base - SPLIT
                        eng.dma_start(
                            out=t1hi[o:o + CCOLS, 0:528]
                            .rearrange("(g p) w -> p g w", p=128),
                            in_=row[:])
                else:
                    hx8 = xp1.tile([128, w], F8, tag="hx8t")
                    nc.sync.dma_start(
                        out=hx8[:],
                        in_=agoutc[k][bass.ds(sregs[r], 128), :])
                    hx = xp1.tile([128, w], F16, tag="hxt")
                    nc.gpsimd.tensor_copy(hx[:], hx8[:])
                    psf = pp1.tile([128, 2, 512], F32, tag="psf")
                    psa = pp1.tile([128, 2, 16], F32, tag="psa")
                    nc.tensor.matmul(psf[0:w, 0, :], hx[:], W1s[:],
                                     start=True, stop=True)
                    nc.tensor.matmul(psa[0:w, 0, :], hx[:], W1as[:],
                                     start=True, stop=True)
                    row = rp1.tile([128, 12, 528], F16, tag="row")
                    nc.scalar.copy(row[0:w, 0, 0:512], psf[0:w, 0, :])
                    nc.vector.tensor_copy(row[0:w, 0, 512:528],
                                          psa[0:w, 0, :])
                    if r < 4:
                        nc.sync.dma_start(out=t1lo[base:base + w, 0:528],
                                          in_=row[0:w, 0, :])
                    else:
                        o = base - SPLIT
                        nc.sync.dma_start(out=t1hi[o:o + w, 0:528],
                                          in_=row[0:w, 0, :])

        # ---------------- phase E: layer-1 edges + epilogue ---------------
        def post1(t, P1, P2, op, pp2, fin):
            zbig, nmxb, seb = fin
            r8 = op.tile([128, 8], F32, tag="r8")
            nc.vector.tensor_scalar_add(r8[:], P2[:], 1e-16)
            nc.vector.reciprocal(r8[:], r8[:])
            nc.vector.tensor_scalar_mul(r8[:], r8[:], 1.0 / HEADS)
            zw = op.tile([128, 512], F32, tag="zw")
            nc.vector.tensor_tensor(
                out=zw[:].rearrange("p (d h) -> p d h", h=HEADS),
                in0=P1[:].rearrange("p (d h) -> p d h", h=HEADS),
                in1=r8[:].unsqueeze(1).broadcast_to([128, 64, HEADS]),
                op=mybir.AluOpType.mult)
            z = zbig[:, t * NCLASS:(t + 1) * NCLASS]
            nc.vector.reduce_sum(
                z, zw[:].rearrange("p (d h) -> p d h", h=HEADS),
                axis=mybir.AxisListType.X)
            nc.vector.tensor_tensor(out=z, in0=z, in1=b1s[:],
                                    op=mybir.AluOpType.add)
            nmx = nmxb[:, t:t + 1]
            nc.vector.reduce_max(nmx, z, axis=mybir.AxisListType.X,
                                 negate=True)
            ez = op.tile([128, NCLASS], F32, tag="ez")
            nc.scalar.activation(ez[:], z,
                                 mybir.ActivationFunctionType.Exp,
                                 bias=nmx, accum_out=seb[:, t:t + 1])

        if "E" in phases:
            fpool = stk.enter_context(tc.tile_pool(name="fin", bufs=1))
            zbig = fpool.tile([128, lt * NCLASS], F32)
            nmxb = fpool.tile([128, lt], F32)
            seb = fpool.tile([128, lt], F32)
            edge_phase(1, post1, (zbig, nmxb, seb))
            # batched log-softmax tail: one Ln + two broadcast ops + 2 DMAs
            nc.scalar.activation(seb[:], seb[:],
                                 mybir.ActivationFunctionType.Ln)
            nc.vector.tensor_tensor(
                out=zbig[:].rearrange("p (t c) -> p t c", c=NCLASS),
                in0=zbig[:].rearrange("p (t c) -> p t c", c=NCLASS),
                in1=nmxb[:].unsqueeze(-1).broadcast_to([128, lt, NCLASS]),
                op=mybir.AluOpType.add)
            nc.vector.tensor_tensor(
                out=zbig[:].rearrange("p (t c) -> p t c", c=NCLASS),
                in0=zbig[:].rearrange("p (t c) -> p t c", c=NCLASS),
                in1=seb[:].unsqueeze(-1).broadcast_to([128, lt, NCLASS]),
                op=mybir.AluOpType.subtract)
            nfull = (lt - 1) * 128
            rlast = LAST_ROWS if lt == LT else 128
            nc.sync.dma_start(
                out=out[0:nfull, :].rearrange("(t p) c -> p t c", p=128),
                in_=zbig[:].rearrange("p (t c) -> p t c", c=NCLASS)
                [:, 0:lt - 1, :])
            nc.sync.dma_start(
                out=out[nfull:nfull + rlast, :],
                in_=zbig[0:rlast, (lt - 1) * NCLASS:lt * NCLASS])

    nc.compile()
    return nc


# --------------------------------------------------------------------------
# entry point
# --------------------------------------------------------------------------

def kernel(**inputs) -> np.ndarray:
    NLs, NHs, in_maps = _prep_inputs(**inputs)
    key = (tuple(NLs), tuple(NHs))
    if key not in _cache:
        _cache[key] = build(NLs, NHs)
    nc = _cache[key]
    res = run_bass_kernel_spmd(nc, in_maps, list(range(NCORES)))
    return np.concatenate([res.results[c]["out"] for c in range(NCORES)], 0)


# revision 13
# speedup vs baseline: 1.2426x; 1.0740x over previous
"""2-layer GAT (nn_GAT_31490700214331) on 8 Trainium2 NeuronCores.

Strategy (dst-sharded, SPMD, per-core-rotated node layout):
  - Nodes are block-partitioned: core c owns nodes [c*6250, (c+1)*6250).
  - Every table on core c uses a ROTATED row order: node n lives at row
    (n - c*6250) mod 50000, so each core's own nodes are rows 0..6249 and
    the single SPMD program has no core-dependent offsets.
  - Layer-0 features (h0 = x @ W0) + attention alphas are computed
    replicated on every core into a rotated f16 DRAM table; edges are
    grouped by dst tile (128 dsts), per-tile chunk counts specialized to
    the actual edge counts (max over cores), and source rows fetched with
    dma_gather through lo/hi table views (int16 indices < 32768).
  - Per-edge dst alphas come from one merged dma_gather over a 256B
    column window of the lo table.
  - Edge softmax (safe without segment-max: |e| <= ~5) and the weighted
    aggregation fuse into per-chunk 128x128 incidence matmuls in PSUM.
  - The ELU'd hidden state is AllGather'd in fp8(e3m4) chunks overlapped
    with phase B, rotated into per-core order, and layer 1 runs on an
    fp8 feature table (f16 alphas riding in the same 768B row) gathered
    at 768B/edge.
  - alpha projections fold into the weight matmuls on the host:
    h @ blockdiag(a) == x @ (W @ blockdiag(a)).

Self-contained: call kernel(**inputs) with the full-problem arrays.
"""
import numpy as np
from contextlib import ExitStack

import concourse.bacc as bacc
import concourse.bass as bass
import concourse.mybir as mybir
from concourse.tile import TileContext
from concourse.bass_utils import run_bass_kernel_spmd

F16 = mybir.dt.float16
F32 = mybir.dt.float32
F8 = mybir.dt.float8e3          # e3m4: 4 mantissa bits, max 15.5
I16 = mybir.dt.int16

N = 50000
NFEAT = 256
NHID = 128
NCLASS = 64
HEADS = 8
SLOPE = 0.2
NCORES = 8
NLOC = N // NCORES           # 6250
LT = (NLOC + 127) // 128     # 49 local dst tiles
LAST_ROWS = NLOC - (LT - 1) * 128   # 106 rows in the last tile
GT = 392                     # global node tiles (392*128 = 50176)
GROWS = GT * 128
SPLIT = 25000                # low/high gather-table split (4 core blocks)
CCOLS = 1536                 # collective chunk width (12 B-tiles)
NCHUNK = 5                   # 4 full chunks + 106-col tail
SENT = 300.0                 # dst_rel sentinel for padding slots
T0W = 256                    # t0 row: [h0(128)|as0(8)|ad0(8)|junk] f16
T1W = 640                    # t1 row: [h1(512)|as1(8)|ad1(8)|junk] f16

_cache = {}


# --------------------------------------------------------------------------
# host-side preparation
# --------------------------------------------------------------------------

def _wrap_idx(idx):
    """[n] int -> [128, n//16] int16 wrapped gather-index layout."""
    n = idx.shape[0]
    assert n % 16 == 0
    w = idx.reshape(n // 16, 16).T.astype(np.int16)
    return np.tile(w, (8, 1))


def _prep_edges(src, dst):
    cores = []
    for c in range(NCORES):
        m = (dst >= c * NLOC) & (dst < (c + 1) * NLOC)
        s = src[m].astype(np.int64)
        d = dst[m].astype(np.int64) - c * NLOC
        order = np.argsort(d, kind="stable")
        s, d = s[order], d[order]
        s_rot = (s - c * NLOC) % N
        tiles = []
        for t in range(LT):
            sel = (d >= t * 128) & (d < (t + 1) * 128)
            st, dt = s_rot[sel], d[sel] - t * 128
            lo = st < SPLIT
            tiles.append((st[lo], dt[lo], st[~lo] - SPLIT, dt[~lo]))
        cores.append(tiles)
    # per-tile chunk counts (max over cores so the SPMD program is shared)
    NLs, NHs = [], []
    for t in range(LT):
        nl = max(len(cores[c][t][0]) for c in range(NCORES))
        nh = max(len(cores[c][t][2]) for c in range(NCORES))
        NLs.append(max(1, (nl + 127) // 128))
        NHs.append(max(1, (nh + 127) // 128))
        assert NLs[t] * 128 <= 1024 and NHs[t] * 128 <= 1024

    out = []
    for c in range(NCORES):
        blocks = []
        for t in range(LT):
            NL, NH = NLs[t], NHs[t]
            CH = NL + NH
            sl, dl, sh, dh = cores[c][t]
            il = np.zeros(NL * 128, np.int64)
            il[: len(sl)] = sl
            ih = np.zeros(NH * 128, np.int64)
            ih[: len(sh)] = sh
            aa = np.zeros(CH * 128, np.int64)
            aa[: len(dl)] = t * 128 + dl
            aa[NL * 128: NL * 128 + len(dh)] = t * 128 + dh
            rl = np.full(NL * 128, SENT)
            rl[: len(dl)] = dl
            rh = np.full(NH * 128, SENT)
            rh[: len(dh)] = dh
            r = np.concatenate([rl, rh]).reshape(CH, 128).T
            drel = np.broadcast_to(
                r.astype(np.float16)[:, :, None],
                (128, CH, 8)).reshape(128, CH * 8)
            blocks.append(np.concatenate(
                [_wrap_idx(il), _wrap_idx(ih), _wrap_idx(aa),
                 np.ascontiguousarray(drel).view(np.int16)], axis=1))
        out.append(dict(epack=np.ascontiguousarray(
            np.concatenate(blocks, axis=1))))
    return NLs, NHs, out


def _prep_inputs(x, edge_index, W0, a_src0, a_dst0, b0, W1, a_src1, a_dst1,
                 b1):
    src = np.asarray(edge_index[0]).astype(np.int64)
    dst = np.asarray(edge_index[1]).astype(np.int64)
    NLs, NHs, edata = _prep_edges(src, dst)

    def bd(a):  # [H, D] -> blockdiag [H*D, H]
        a = np.asarray(a, np.float32)
        H, D = a.shape
        m = np.zeros((H * D, H), np.float32)
        for h in range(H):
            m[h * D:(h + 1) * D, h] = a[h]
        return m

    W0 = np.asarray(W0, np.float32)
    W1 = np.asarray(W1, np.float32)
    W0a = np.concatenate([W0 @ bd(a_src0), W0 @ bd(a_dst0)], 1)  # [256, 16]
    # head-innermost feature interleave: new col d*8+h <- old col h*D+d
    perm0 = np.array([(f % 8) * 16 + f // 8 for f in range(128)])
    perm1 = np.array([(f % 8) * 64 + f // 8 for f in range(512)])
    W0cat = np.concatenate([W0[:, perm0], W0a], 1)               # [256, 144]
    W1a = np.concatenate([W1 @ bd(a_src1), W1 @ bd(a_dst1)], 1)  # [128, 16]

    x = np.asarray(x, np.float32)
    ident = np.eye(128, dtype=np.float16)
    colio = np.tile(np.arange(128, dtype=np.float16)[None, :], (128, 1))
    b0b = np.tile(np.asarray(b0, np.float32)[None, :], (128, 1))
    b1b = np.tile(np.asarray(b1, np.float32)[None, :], (128, 1))

    in_maps = []
    for c in range(NCORES):
        rot = np.roll(np.arange(N), -c * NLOC)
        xr = np.zeros((GROWS, NFEAT), np.float16)
        xr[:N] = x[rot].astype(np.float16)
        # [gg, 128(j feat), 2(g), 2(k), 128(p node)]: partition = feature,
        # per-partition contiguous 1KB runs
        xtt = (xr.reshape(GROWS // 256, 2, 128, 2, 128)
               .transpose(0, 4, 1, 3, 2))
        m = dict(
            xT=np.ascontiguousarray(xtt),
            W0=np.ascontiguousarray(
                W0cat.astype(np.float16).reshape(2, 128, NHID + 16)),
            W1=np.ascontiguousarray(W1[perm0][:, perm1].astype(np.float16)),
            W1a=np.ascontiguousarray(W1a[perm0].astype(np.float16)),
            b0b=np.ascontiguousarray(b0b[:, perm0]), b1b=b1b,
            ident=ident, colio=colio,
            **edata[c],
        )
        in_maps.append(m)
    return NLs, NHs, in_maps


# --------------------------------------------------------------------------
# device program
# --------------------------------------------------------------------------

def build(NLs, NHs, lt=LT, gt=GT, debug=False, phases="ABCDE",
          sim_safe=False):
    NLs, NHs = list(NLs), list(NHs)
    HID16 = NHID + 16
    EPW = [(NLs[t] + NHs[t]) * 24 for t in range(lt)]
    EOFF = np.concatenate([[0], np.cumsum(EPW)]).astype(int)
    nc = bacc.Bacc("TRN2")
    xT = nc.dram_tensor("xT", [GROWS // 256, 128, 2, 2, 128], F16,
                        kind="ExternalInput")
    W0i = nc.dram_tensor("W0", [2, 128, NHID + 16], F16,
                         kind="ExternalInput")
    W1i = nc.dram_tensor("W1", [NHID, 512], F16, kind="ExternalInput")
    W1ai = nc.dram_tensor("W1a", [NHID, 16], F16, kind="ExternalInput")
    b0bi = nc.dram_tensor("b0b", [128, NHID], F32, kind="ExternalInput")
    b1bi = nc.dram_tensor("b1b", [128, NCLASS], F32, kind="ExternalInput")
    identi = nc.dram_tensor("ident", [128, 128], F16, kind="ExternalInput")
    colioi = nc.dram_tensor("colio", [128, 128], F16, kind="ExternalInput")
    epacki = nc.dram_tensor("epack", [128, int(EOFF[-1])], I16,
                            kind="ExternalInput")
    out = nc.dram_tensor("out", [NLOC, NCLASS], F32, kind="ExternalOutput")

    with TileContext(nc) as tc, ExitStack() as stk:
        regs = {}

        def reg_of(n):
            if n not in regs:
                regs[n] = nc.gpsimd.to_reg(n)
            return regs[n]

        dpool = stk.enter_context(
            tc.tile_pool(name="dram", bufs=1, space="DRAM"))
        t0lo = dpool.tile([SPLIT, T0W], F16, tag="t0lo")
        t0hi = dpool.tile([GROWS - SPLIT, T0W], F16, tag="t0hi")
        t1lo = dpool.tile([SPLIT, T1W], F16, tag="t1lo")
        t1hi = dpool.tile([GROWS - SPLIT, T1W], F16, tag="t1hi")
        CW = [CCOLS] * 4 + [NLOC - 4 * CCOLS]   # 4x1536 + 106
        aginc = [dpool.tile([128, CW[k]], F8, tag=f"agin{k}",
                            name=f"agin{k}")
                 for k in range(NCHUNK)]
        agoutc = [dpool.tile([NCORES * 128, CW[k]], F8, tag=f"agout{k}",
                             addr_space="Shared", name=f"agout{k}")
                  for k in range(NCHUNK)]

        cpool = stk.enter_context(tc.tile_pool(name="const", bufs=1))
        W0s = cpool.tile([128, 2, NHID + 16], F16)
        nc.sync.dma_start(out=W0s[:], in_=W0i.rearrange("k p n -> p k n"))
        W1s = cpool.tile([128, 512], F16)
        nc.sync.dma_start(out=W1s[:], in_=W1i[:])
        W1as = cpool.tile([128, 16], F16)
        nc.sync.dma_start(out=W1as[:], in_=W1ai[:])
        b0s = cpool.tile([128, NHID], F32)
        nc.sync.dma_start(out=b0s[:], in_=b0bi[:])
        b1s = cpool.tile([128, NCLASS], F32)
        nc.sync.dma_start(out=b1s[:], in_=b1bi[:])
        idents = cpool.tile([128, 128], F16)
        nc.sync.dma_start(out=idents[:], in_=identi[:])
        colios = cpool.tile([128, 128], F16)
        nc.sync.dma_start(out=colios[:], in_=colioi[:])

        pid = nc.partition_id(engines=[mybir.EngineType.SP])
        sregs = [nc.sync.snap(((j + pid) % NCORES) * 128)
                 for j in range(NCORES)]

        # ---------------- phase A: layer-0 tables (replicated) ------------
        with ExitStack() as pa:
            xp = pa.enter_context(tc.tile_pool(name="pa_x", bufs=4))
            pp = pa.enter_context(
                tc.tile_pool(name="pa_ps", bufs=2, space="PSUM"))
            rp = pa.enter_context(tc.tile_pool(name="pa_row", bufs=4))
            assert gt % 4 == 0
            for gq in range(gt // 4):
                # two 2-group units per load to halve DMA issue count
                xa = xp.tile([128, 2, 2, 2, 128], F16, tag="xa")
                leng = nc.sync if gq % 2 else nc.gpsimd
                leng.dma_start(
                    out=xa[:],
                    in_=xT[2 * gq:2 * gq + 2]
                    .rearrange("G p g k f -> p G g k f"))
                for G in range(2):
                    gg = 2 * gq + G
                    row = rp.tile([128, 2, T0W], F16, tag="row")
                    for g2 in range(2):
                        ps = pp.tile([128, HID16], F32, tag=f"ps{g2}")
                        for k in range(2):
                            nc.tensor.matmul(ps[:], xa[:, G, g2, k, :],
                                             W0s[:, k, :],
                                             start=(k == 0), stop=(k == 1))
                        eng2 = nc.vector.tensor_copy if g2 else nc.scalar.copy
                        eng2(row[:, g2, 0:HID16], ps[:])
                    eng = nc.gpsimd if gg % 2 else nc.sync
                    g0 = 2 * gg * 128
                    if g0 + 256 <= SPLIT:
                        eng.dma_start(
                            out=t0lo[g0:g0 + 256, 0:HID16]
                            .rearrange("(g p) w -> p g w", p=128),
                            in_=row[:, :, 0:HID16])
                    elif g0 >= SPLIT:
                        o = g0 - SPLIT
                        eng.dma_start(
                            out=t0hi[o:o + 256, 0:HID16]
                            .rearrange("(g p) w -> p g w", p=128),
                            in_=row[:, :, 0:HID16])
                    else:
                        # group straddles the lo/hi split inside tile g2=1
                        cut = SPLIT - g0 - 128
                        eng.dma_start(out=t0lo[g0:g0 + 128, 0:HID16],
                                      in_=row[:, 0, 0:HID16])
                        eng.dma_start(
                            out=t0lo[g0 + 128:SPLIT, 0:HID16],
                            in_=row[0:cut, 1, 0:HID16])
                        eng.dma_start(
                            out=t0hi[0:256 - 128 - cut, 0:HID16],
                            in_=row[cut:128, 1, 0:HID16])

        # ---------------- shared edge phase -------------------------------
        def edge_phase(layer, post_fn, fin, hook=None):
            if layer == 0:
                tbl_lo, tbl_hi, trow, fdim = t0lo, t0hi, T0W, NHID
                gdt, adt, awcols = F16, F16, 128
                awin = t0lo[:, 128:256]
            else:
                tbl_lo, tbl_hi, trow, fdim = t1lo, t1hi, T1W, 512
                gdt, adt, awcols = F16, F16, 128
                awin = t1lo[:, 512:640]
            D = fdim // HEADS
            with ExitStack() as pb:
                ip = pb.enter_context(
                    tc.tile_pool(name=f"ix{layer}", bufs=4))
                gp = pb.enter_context(
                    tc.tile_pool(name=f"gg{layer}", bufs=4))
                apl = pb.enter_context(
                    tc.tile_pool(name=f"ga{layer}", bufs=3))
                rp2 = pb.enter_context(
                    tc.tile_pool(name=f"rh{layer}", bufs=3))
                pp2 = pb.enter_context(
                    tc.tile_pool(name=f"ps{layer}", bufs=2, space="PSUM"))
                op = pb.enter_context(
                    tc.tile_pool(name=f"po{layer}", bufs=3))
                for t in range(lt):
                    NL, NH = NLs[t], NHs[t]
                    CH = NL + NH
                    NLI, NHI = NL * 128, NH * 128
                    o_ih = NL * 8
                    o_ea = CH * 8
                    o_dr = CH * 16
                    ep = ip.tile([128, EPW[t]], I16, tag="ep")
                    nc.sync.dma_start(
                        out=ep[:], in_=epacki[:, EOFF[t]:EOFF[t + 1]])
                    il = ep[:, 0:NL * 8]
                    ih = ep[:, o_ih:o_ih + NH * 8]
                    ea = ep[:, o_ea:o_ea + CH * 8]
                    dr8 = (ep[:, o_dr:o_dr + CH * 8].bitcast(F16)
                           .rearrange("p (c e) -> p c e", e=8))

                    G = gp.tile([128, CH, trow], gdt, tag="G")
                    nc.gpsimd.dma_gather(G[:, 0:NL, :], tbl_lo[:], il,
                                         NLI, reg_of(NLI), trow,
                                         elem_step=trow)
                    nc.gpsimd.dma_gather(G[:, NL:CH, :], tbl_hi[:],
                                         ih, NHI, reg_of(NHI), trow,
                                         elem_step=trow)
                    # dst-alpha gathers: 256B column window of the lo table
                    # (<=1024 idx per SWDGE call)
                    A = apl.tile([128, CH, awcols], adt, tag="A")
                    nc.gpsimd.dma_gather(A[:, 0:NL, :], awin,
                                         ea[:, 0:NL * 8], NLI, reg_of(NLI),
                                         awcols, elem_step=trow)
                    nc.gpsimd.dma_gather(A[:, NL:CH, :], awin,
                                         ea[:, NL * 8:CH * 8], NHI,
                                         reg_of(NHI), awcols, elem_step=trow)
                    g_as = G[:, :, fdim:fdim + 8]
                    a_ad = A[:, :, 8:16]
                    g_f = G[:, :, 0:fdim]

                    inc = rp2.tile([128, CH, 128], F16, tag="inc")
                    nc.vector.tensor_tensor(
                        out=inc[:].rearrange("p c (g e) -> p c g e", e=8),
                        in0=dr8.unsqueeze(2)
                        .broadcast_to([128, CH, 16, 8]),
                        in1=colios[:].rearrange("p (g e) -> p g e", e=8)
                        .unsqueeze(1).broadcast_to([128, CH, 16, 8]),
                        op=mybir.AluOpType.is_equal)
                    EX = rp2.tile([128, CH, 8], F16, tag="EX")
                    nc.vector.tensor_tensor(
                        out=EX[:], in0=g_as, in1=a_ad,
                        op=mybir.AluOpType.add)
                    if sim_safe:
                        EXr = rp2.tile([128, CH, 8], F16, tag="EXr")
                        nc.scalar.activation(
                            EXr[:], EX[:],
                            mybir.ActivationFunctionType.Relu, scale=0.8)
                        nc.vector.tensor_scalar_mul(EX[:], EX[:], SLOPE)
                        nc.vector.tensor_tensor(
                            out=EX[:], in0=EX[:], in1=EXr[:],
                            op=mybir.AluOpType.add)
                    else:
                        nc.scalar.activation(
                            EX[:], EX[:],
                            mybir.ActivationFunctionType.Prelu, alpha=SLOPE)
                    nc.scalar.activation(
                        EX[:], EX[:], mybir.ActivationFunctionType.Exp)

                    R = rp2.tile([128, CH, fdim], F16, tag="R")
                    nc.vector.tensor_tensor(
                        out=R[:].rearrange("p c (d h) -> p c d h", h=HEADS),
                        in0=g_f.rearrange("p c (d h) -> p c d h", h=HEADS),
                        in1=EX[:].unsqueeze(2)
                        .broadcast_to([128, CH, D, HEADS]),
                        op=mybir.AluOpType.mult)

                    P1 = pp2.tile([128, fdim], F32, tag="P1")
                    P2 = pp2.tile([128, 8], F32, tag="P2")
                    for ch in range(CH):
                        nc.tensor.matmul(P1[:], inc[:, ch, :],
                                         R[:, ch, 0:fdim],
                                         start=(ch == 0),
                                         stop=(ch == CH - 1))
                    for ch in range(CH):
                        nc.tensor.matmul(P2[:], inc[:, ch, :],
                                         EX[:, ch, :],
                                         start=(ch == 0),
                                         stop=(ch == CH - 1))
                    post_fn(t, P1, P2, op, pp2, fin)
                    if hook is not None:
                        hook(t)

        # ---- L0 post: softmax-div, +b0, ELU, transpose, store ------------
        def post0(t, P1, P2, op, pp2, fin):
            rows = 128 if t < lt - 1 else LAST_ROWS
            r8 = op.tile([128, 8], F32, tag="r8")
            nc.vector.tensor_scalar_add(r8[:], P2[:], 1e-16)
            nc.vector.reciprocal(r8[:], r8[:])
            z = op.tile([128, NHID], F32, tag="z")
            nc.vector.tensor_tensor(
                out=z[:].rearrange("p (d h) -> p d h", h=HEADS),
                in0=P1[:].rearrange("p (d h) -> p d h", h=HEADS),
                in1=r8[:].unsqueeze(1).broadcast_to([128, 16, HEADS]),
                op=mybir.AluOpType.mult)
            nc.vector.tensor_tensor(out=z[:], in0=z[:], in1=b0s[:],
                                    op=mybir.AluOpType.add)
            zm = op.tile([128, NHID], F32, tag="zm")
            nc.vector.tensor_scalar_min(zm[:], z[:], 0.0)
            nc.scalar.activation(zm[:], zm[:],
                                 mybir.ActivationFunctionType.Exp)
            zp = op.tile([128, NHID], F32, tag="zp")
            nc.vector.tensor_scalar_max(zp[:], z[:], 0.0)
            nc.vector.tensor_tensor(out=zp[:], in0=zp[:], in1=zm[:],
                                    op=mybir.AluOpType.add)
            h1 = op.tile([128, NHID], F16, tag="h1")
            nc.vector.tensor_scalar_add(h1[:], zp[:], -1.0)
            pst = pp2.tile([128, 128], F16, tag="pst")
            nc.tensor.transpose(pst[:], h1[:], idents[:])
            hT = op.tile([128, 128], F8, tag="hT")
            nc.vector.tensor_copy(hT[:], pst[:])
            k = min(t // 12, NCHUNK - 1)
            col = (t - k * 12) * 128
            nc.sync.dma_start(
                out=aginc[k][:, col:col + rows], in_=hT[:, 0:rows])

        # chunked AllGather: issued from inside the B loop as soon as a
        # chunk's 12 tiles land, overlapping the collective with B and D
        def issue_coll(k):
            nc.gpsimd.collective_compute(
                "AllGather", mybir.AluOpType.bypass,
                replica_groups=[list(range(NCORES))],
                ins=[aginc[k][:]], outs=[agoutc[k][:]])

        if "B" in phases:
            edge_phase(0, post0, None)
            if "C" in phases:
                for k in range(NCHUNK):
                    issue_coll(k)

        # ---------------- phase D: layer-1 tables (chunk-major) -----------
        with ExitStack() as pd:
            xp1 = pd.enter_context(tc.tile_pool(name="pd_x", bufs=4))
            pp1 = pd.enter_context(
                tc.tile_pool(name="pd_ps", bufs=2, space="PSUM"))
            rp1 = pd.enter_context(tc.tile_pool(name="pd_row", bufs=2))
            dunits = ([(k, r) for k in range(NCHUNK) for r in range(NCORES)]
                      if "D" in phases else [])
            wengs = [nc.sync, nc.scalar]
            for k, r in dunits:
                base = r * NLOC + k * CCOLS
                w = CW[k]
                if w == CCOLS:
                    hx8 = xp1.tile([128, CCOLS], F8, tag="hx8")
                    nc.sync.dma_start(
                        out=hx8[:],
                        in_=agoutc[k][bass.ds(sregs[r], 128), :])
                    hx = xp1.tile([128, CCOLS], F16, tag="hx")
                    if (k * NCORES + r) % 2:
                        nc.vector.tensor_copy(hx[:], hx8[:])
                    else:
                        nc.scalar.copy(hx[:], hx8[:])
                    row = rp1.tile([128, 12, 528], F16, tag="row")
                    for g2 in range(6):
                        # each q's 512-col matmul exactly fills one PSUM
                        # bank (outputs must not cross 2KB banks)
                        psf = pp1.tile([128, 2, 512], F32, tag="psf")
                        psa = pp1.tile([128, 2, 16], F32, tag="psa")
                        for q in range(2):
                            hs = hx[:, (g2 * 2 + q) * 128:
                                    (g2 * 2 + q + 1) * 128]
                            nc.tensor.matmul(psf[:, q, :], hs, W1s[:],
                                             start=True, stop=True)
                            nc.tensor.matmul(psa[:, q, :], hs,
                                             W1as[:], start=True, stop=True)
                        feng = nc.scalar.copy if g2 % 2 else \
                            nc.vector.tensor_copy
                        aeng = nc.vector.tensor_copy if g2 % 2 else \
                            nc.scalar.copy
                        feng(row[:, 2 * g2:2 * g2 + 2, 0:512], psf[:])
                        aeng(row[:, 2 * g2:2 * g2 + 2, 512:528], psa[:])
                    eng = wengs[(k * NCORES + r) % 2]
                    if r < 4:
                        eng.dma_start(
                            out=t1lo[base:base + CCOLS, 0:528]
                            .rearrange("(g p) w -> p g w", p=128),
                            in_=row[:])
                    else:
                        o = base - SPLIT
                        eng.dma_start(
                            out=t1hi[o:o + CCOLS, 0:528]
                            .rearrange("(g p) w -> p g w", p=128),
                            in_=row[:])
                else:
                    hx8 = xp1.tile([128, w], F8, tag="hx8t")
                    nc.sync.dma_start(
                        out=hx8[:],
                        in_=agoutc[k][bass.ds(sregs[r], 128), :])
                    hx = xp1.tile([128, w], F16, tag="hxt")
                    nc.scalar.copy(hx[:], hx8[:])
                    psf = pp1.tile([128, 2, 512], F32, tag="psf")
                    psa = pp1.tile([128, 2, 16], F32, tag="psa")
                    nc.tensor.matmul(psf[0:w, 0, :], hx[:], W1s[:],
                                     start=True, stop=True)
                    nc.tensor.matmul(psa[0:w, 0, :], hx[:], W1as[:],
                                     start=True, stop=True)
                    row = rp1.tile([128, 12, 528], F16, tag="row")
                    nc.scalar.copy(row[0:w, 0, 0:512], psf[0:w, 0, :])
                    nc.vector.tensor_copy(row[0:w, 0, 512:528],
                                          psa[0:w, 0, :])
                    if r < 4:
                        nc.sync.dma_start(out=t1lo[base:base + w, 0:528],
                                          in_=row[0:w, 0, :])
                    else:
                        o = base - SPLIT
                        nc.sync.dma_start(out=t1hi[o:o + w, 0:528],
                                          in_=row[0:w, 0, :])

        # ---------------- phase E: layer-1 edges + epilogue ---------------
        def post1(t, P1, P2, op, pp2, fin):
            zbig, nmxb, seb = fin
            r8 = op.tile([128, 8], F32, tag="r8")
            nc.vector.tensor_scalar_add(r8[:], P2[:], 1e-16)
            nc.vector.reciprocal(r8[:], r8[:])
            nc.vector.tensor_scalar_mul(r8[:], r8[:], 1.0 / HEADS)
            zw = op.tile([128, 512], F32, tag="zw")
            nc.vector.tensor_tensor(
                out=zw[:].rearrange("p (d h) -> p d h", h=HEADS),
                in0=P1[:].rearrange("p (d h) -> p d h", h=HEADS),
                in1=r8[:].unsqueeze(1).broadcast_to([128, 64, HEADS]),
                op=mybir.AluOpType.mult)
            z = zbig[:, t * NCLASS:(t + 1) * NCLASS]
            nc.vector.reduce_sum(
                z, zw[:].rearrange("p (d h) -> p d h", h=HEADS),
                axis=mybir.AxisListType.X)
            nc.vector.tensor_tensor(out=z, in0=z, in1=b1s[:],
                                    op=mybir.AluOpType.add)
            nmx = nmxb[:, t:t + 1]
            nc.vector.reduce_max(nmx, z, axis=mybir.AxisListType.X,
                                 negate=True)
            ez = op.tile([128, NCLASS], F32, tag="ez")
            nc.scalar.activation(ez[:], z,
                                 mybir.ActivationFunctionType.Exp,
                                 bias=nmx, accum_out=seb[:, t:t + 1])

        if "E" in phases:
            fpool = stk.enter_context(tc.tile_pool(name="fin", bufs=1))
            zbig = fpool.tile([128, lt * NCLASS], F32)
            nmxb = fpool.tile([128, lt], F32)
            seb = fpool.tile([128, lt], F32)
            edge_phase(1, post1, (zbig, nmxb, seb))
            # batched log-softmax tail: one Ln + two broadcast ops + 2 DMAs
            nc.scalar.activation(seb[:], seb[:],
                                 mybir.ActivationFunctionType.Ln)
            nc.vector.tensor_tensor(
                out=zbig[:].rearrange("p (t c) -> p t c", c=NCLASS),
                in0=zbig[:].rearrange("p (t c) -> p t c", c=NCLASS),
                in1=nmxb[:].unsqueeze(-1).broadcast_to([128, lt, NCLASS]),
                op=mybir.AluOpType.add)
            nc.vector.tensor_tensor(
                out=zbig[:].rearrange("p (t c) -> p t c", c=NCLASS),
                in0=zbig[:].rearrange("p (t c) -> p t c", c=NCLASS),
                in1=seb[:].unsqueeze(-1).broadcast_to([128, lt, NCLASS]),
                op=mybir.AluOpType.subtract)
            nfull = (lt - 1) * 128
            rlast = LAST_ROWS if lt == LT else 128
            nc.sync.dma_start(
                out=out[0:nfull, :].rearrange("(t p) c -> p t c", p=128),
                in_=zbig[:].rearrange("p (t c) -> p t c", c=NCLASS)
                [:, 0:lt - 1, :])
            nc.sync.dma_start(
                out=out[nfull:nfull + rlast, :],
                in_=zbig[0:rlast, (lt - 1) * NCLASS:lt * NCLASS])

    nc.compile()
    return nc


# --------------------------------------------------------------------------
# entry point
# --------------------------------------------------------------------------

def kernel(**inputs) -> np.ndarray:
    NLs, NHs, in_maps = _prep_inputs(**inputs)
    key = (tuple(NLs), tuple(NHs))
    if key not in _cache:
        _cache[key] = build(NLs, NHs)
    nc = _cache[key]
    res = run_bass_kernel_spmd(nc, in_maps, list(range(NCORES)))
    return np.concatenate([res.results[c]["out"] for c in range(NCORES)], 0)
